# revision 1
# baseline (speedup 1.0000x reference)
"""XL-BOMD rank-4 Krylov propagation (EnergyXL) on 8 TRN2 NeuronCores.

Data-parallel over molecules: 512 mols -> 64 per core. Per molecule
(N=192, rank=4):
  dDS = D - P ; v0 = dDS/||dDS||
  for k in 0..3:  T = v_k R ; W_k = R T - v_k   (PE sandwiches, fp32)
                  v_{k+1} = GS-orthonormalize(W_k vs v_0..v_k)
  O[IJ] = <W_I,W_J>, c[J] = <W_J,dDS>  (Frobenius ips, DVE TTR)
  y = O^-1 c (batched symmetric Gauss elim over mol-partition layout)
  out = -sum_I y_I V_I

Matrices live in SBUF as hi [128,192] (rows 0:128) + lo [64,192]
(rows 128:192) fp32 tiles.  Inner products use fused
tensor_tensor_reduce with hi->lo accumulator chaining; cross-partition
sums + per-mol scalar broadcast via ones-matmul on the PE.
"""

import os
import sys

sys.path.insert(0, "/opt/trn_rl_repo")

import numpy as np

import concourse.bass as bass
import concourse.bacc as bacc
import concourse.tile as tile
from concourse import mybir
from concourse.bass_utils import run_bass_kernel_spmd

F32 = mybir.dt.float32
ALU = mybir.AluOpType
ACTF = mybir.ActivationFunctionType

NMOL, N, RANK = 512, 192, 4
NCORES = 8
MPC = NMOL // NCORES  # 64 molecules per core
HI, LO = 128, 64
BLK = 8  # molecules per solve block

# partials column map (per molecule, [128, 48] tile); every quantity is a
# (hi, lo) column pair summed post-broadcast (strided pair-add).
# Layout is rank-contiguous so ONE ones-matmul per rank broadcasts
# O_kk + c_k + GS coefs together:
#   col 0,1: ||dDS||^2
#   rank-k block at RBASE[k]: O(k,0..k) pairs, c_k pair, GS(k+1, 0..k) pairs
COL_N0 = 0
RBASE = [2, 8, 18, 32]          # rank block bases; sizes 6,10,14,10
NPART = 48
# post-gather pair-added column indices (gather covers cols 2..41 -> 20 pairs)
GIX = {"O00": 0, "c0": 1, "G10": 2, "O10": 3, "O11": 4, "c1": 5, "G20": 6,
       "G21": 7, "O20": 8, "O21": 9, "O22": 10, "c2": 11, "G30": 12,
       "G31": 13, "G32": 14, "O30": 15, "O31": 16, "O32": 17, "O33": 18,
       "c3": 19}


def _o_col(k, j):
    assert j <= k
    return RBASE[k] + 2 * j


def _c_col(k):
    return RBASE[k] + 2 * (k + 1)


def _gs_col(kk, j):
    # coef <w_k, v_j> for v_kk (kk = k+1), stored in rank-k block
    k = kk - 1
    return RBASE[k] + 2 * (k + 1) + 2 + 2 * j


def build_core_kernel(n_mols=MPC):
    nc = bacc.Bacc(None, target_bir_lowering=False, enable_partition_id=False)
    D = nc.dram_tensor("D", [n_mols, N, N], F32, kind="ExternalInput")
    P = nc.dram_tensor("P", [n_mols, N, N], F32, kind="ExternalInput")
    R = nc.dram_tensor("Rm", [n_mols, N, N], F32, kind="ExternalInput")
    OUT = nc.dram_tensor("OUT", [n_mols, N, N], F32, kind="ExternalOutput")

    with tile.TileContext(nc) as tc:
        _body(nc, tc, D, P, R, OUT)
    nc.finalize()
    return nc


def _body(nc, tc, D, P, R, OUT):
    import contextlib

    ctx = contextlib.ExitStack()
    with ctx:
        consts = ctx.enter_context(tc.tile_pool(name="consts", bufs=1))
        persist = ctx.enter_context(tc.tile_pool(name="persist", bufs=11))
        fast = ctx.enter_context(tc.tile_pool(name="fast", bufs=6))
        temps = ctx.enter_context(tc.tile_pool(name="temps", bufs=8))
        scal = ctx.enter_context(tc.tile_pool(name="scal", bufs=16))
        blkp = ctx.enter_context(tc.tile_pool(name="blkp", bufs=2))
        ps_mm = ctx.enter_context(tc.tile_pool(name="ps_mm", bufs=2, space="PSUM"))
        ps_sm = ctx.enter_context(tc.tile_pool(name="ps_sm", bufs=2, space="PSUM"))
        ps_g = ctx.enter_context(tc.tile_pool(name="ps_g", bufs=2, space="PSUM"))

        # --- constants ---
        ones = consts.tile([HI, HI], F32)      # all-ones for partition-sum / bcast matmuls
        nc.vector.memset(ones, 1.0)
        sel = consts.tile([HI, 2 * LO - 1], F32)  # windowed one-hot column selector
        nc.vector.memset(sel, 0.0)
        nc.vector.memset(sel[:, LO - 1 : LO], 1.0)
        id8 = consts.tile([BLK, BLK], F32)     # identity for y row-masking
        idt = consts.tile([BLK, BLK], mybir.dt.int32)
        nc.gpsimd.iota(idt, pattern=[[-1, BLK]], base=0, channel_multiplier=1)
        nc.vector.tensor_scalar(out=id8, in0=idt, scalar1=0, scalar2=None,
                                op0=ALU.is_equal)

        n_mols = D.shape[0]
        for b in range(n_mols // BLK):
            mols = list(range(b * BLK, (b + 1) * BLK))
            blk_state = []
            for m in mols:
                st = _mol_pipeline(nc, tc, D, P, R, m, persist, fast, temps, scal,
                                   ps_mm, ps_sm, ones)
                blk_state.append(st)
            _block_tail(nc, tc, OUT, b, mols, blk_state, consts, fast, temps, scal,
                        blkp, ps_g, ps_sm, ones, sel, id8)


def _ip(nc, partials, col2, a, b_, scr, scr2, mult_eng="dve", red="act"):
    """<A,B> Frobenius: elementwise mult then free-dim reduce into col pair."""
    a_hi, a_lo = a
    b_hi, b_lo = b_
    scr_hi, scr_lo = scr
    me = nc.vector if mult_eng == "dve" else nc.gpsimd
    me.tensor_mul(scr_hi, a_hi, b_hi)
    me.tensor_mul(scr_lo, a_lo, b_lo)
    if red == "act":
        nc.scalar.activation(out=scr_hi, in_=scr_hi, func=ACTF.Copy,
                             accum_out=partials[:, col2 : col2 + 1])
        nc.scalar.activation(out=scr_lo, in_=scr_lo, func=ACTF.Copy,
                             accum_out=partials[:LO, col2 + 1 : col2 + 2])
    else:
        nc.vector.tensor_reduce(out=partials[:, col2 : col2 + 1], in_=scr_hi,
                                axis=mybir.AxisListType.X, op=ALU.add)
        nc.vector.tensor_reduce(out=partials[:LO, col2 + 1 : col2 + 2], in_=scr_lo,
                                axis=mybir.AxisListType.X, op=ALU.add)


def _norm_sq(nc, partials, col2, x, scr_hi, scr_lo):
    """||X||^2 on ACT: square+accumulate, hi/lo to cols col2, col2+1."""
    x_hi, x_lo = x
    nc.scalar.activation(out=scr_hi, in_=x_hi, func=ACTF.Square,
                         accum_out=partials[:, col2 : col2 + 1])
    nc.scalar.activation(out=scr_lo, in_=x_lo, func=ACTF.Square,
                         accum_out=partials[:LO, col2 + 1 : col2 + 2])


def _bcast(nc, ps_sm, ones, partials, col, ncols):
    """ones-matmul: col sums of partials[:, col:col+ncols] broadcast to 128 partitions."""
    bc = ps_sm.tile([HI, ncols], F32, tag="bc")
    nc.tensor.matmul(bc, lhsT=ones, rhs=partials[:, col : col + ncols],
                     start=True, stop=True)
    return bc


def _inv_norm(nc, scal, ps_sm, ones, partials, col2, tag):
    """1/sqrt(hi_col + lo_col) as a [128,1] SBUF tile."""
    bc = _bcast(nc, ps_sm, ones, partials, col2, 2)
    s = scal.tile([HI, 5], F32, tag=tag)
    nc.scalar.copy(s[:, 0:2], bc)
    nc.vector.tensor_add(s[:, 2:3], s[:, 0:1], s[:, 1:2])
    nc.scalar.sqrt(s[:, 3:4], s[:, 2:3])
    nc.vector.reciprocal(s[:, 4:5], s[:, 3:4])
    return s[:, 4:5]


def _sandwich(nc, ps_mm, out_sb, lhsT, rhs):
    """out = lhsT^T @ rhs for 192x192 operands in hi/lo tiles -> PSUM pair."""
    l_hi, l_lo = lhsT
    r_hi, r_lo = rhs
    o_hi = ps_mm.tile([HI, N], F32, tag="mm_hi")
    o_lo = ps_mm.tile([LO, N], F32, tag="mm_lo")
    nc.tensor.matmul(o_hi, lhsT=l_hi[:, 0:HI], rhs=r_hi, start=True, stop=False)
    nc.tensor.matmul(o_hi, lhsT=l_lo[:, 0:HI], rhs=r_lo, start=False, stop=True)
    nc.tensor.matmul(o_lo, lhsT=l_hi[:, HI:N], rhs=r_hi, start=True, stop=False)
    nc.tensor.matmul(o_lo, lhsT=l_lo[:, HI:N], rhs=r_lo, start=False, stop=True)
    return o_hi, o_lo


def _mol_pipeline(nc, tc, D, P, R, m, persist, fast, temps, scal, ps_mm, ps_sm, ones):
    """Emit one molecule's Krylov chain. Returns state dict for the block tail."""
    # --- load R, D, P ---
    r_hi = fast.tile([HI, N], F32, tag="r_hi")
    r_lo = fast.tile([LO, N], F32, tag="r_lo")
    nc.sync.dma_start(out=r_hi, in_=R[m, 0:HI, :])
    nc.sync.dma_start(out=r_lo, in_=R[m, HI:N, :])

    dds_hi = persist.tile([HI, N], F32, tag="dds_hi")
    dds_lo = persist.tile([LO, N], F32, tag="dds_lo")
    nc.sync.dma_start(out=dds_hi, in_=D[m, 0:HI, :])
    nc.sync.dma_start(out=dds_lo, in_=D[m, HI:N, :])
    p_hi = temps.tile([HI, N], F32, tag="p_hi")
    p_lo = temps.tile([LO, N], F32, tag="p_lo")
    nc.sync.dma_start(out=p_hi, in_=P[m, 0:HI, :])
    nc.sync.dma_start(out=p_lo, in_=P[m, HI:N, :])
    nc.gpsimd.tensor_sub(dds_hi, dds_hi, p_hi)
    nc.gpsimd.tensor_sub(dds_lo, dds_lo, p_lo)
    dds = (dds_hi, dds_lo)

    partials = scal.tile([HI, NPART], F32, tag="partials")
    nc.vector.memset(partials, 0.0)

    scr_hi = temps.tile([HI, N], F32, tag="scr_hi")
    scr_lo = temps.tile([LO, N], F32, tag="scr_lo")
    scr = (scr_hi, scr_lo)
    scr2 = None

    # --- v0 = dDS / ||dDS|| ---
    _norm_sq(nc, partials, COL_N0, dds, scr_hi, scr_lo)
    inv0 = _inv_norm(nc, scal, ps_sm, ones, partials, COL_N0, tag="nrm0")
    v_tiles = []
    v0_hi = persist.tile([HI, N], F32, tag="v0_hi")
    v0_lo = persist.tile([LO, N], F32, tag="v0_lo")
    nc.scalar.mul(v0_hi, dds_hi, inv0)
    nc.scalar.mul(v0_lo, dds_lo, inv0[:LO, :])
    v_tiles.append((v0_hi, v0_lo))

    w_tiles = []
    for k in range(RANK):
        vk = v_tiles[k]
        # T = v_k R
        t_ps = _sandwich(nc, ps_mm, None, vk, (r_hi, r_lo))
        t_hi = temps.tile([HI, N], F32, tag="t_hi")
        t_lo = temps.tile([LO, N], F32, tag="t_lo")
        nc.scalar.copy(t_hi, t_ps[0])
        nc.scalar.copy(t_lo, t_ps[1])
        # W_k = R T - v_k
        w_ps = _sandwich(nc, ps_mm, None, (r_hi, r_lo), (t_hi, t_lo))
        w_hi = fast.tile([HI, N], F32, tag=f"w{k}_hi")
        w_lo = fast.tile([LO, N], F32, tag=f"w{k}_lo")
        nc.vector.tensor_sub(w_hi, w_ps[0], vk[0])
        nc.vector.tensor_sub(w_lo, w_ps[1], vk[1])
        wk = (w_hi, w_lo)
        w_tiles.append(wk)

        # O row k and c_k  (off the critical chain)
        for j in range(k + 1):
            _ip(nc, partials, _o_col(k, j), w_tiles[j], wk, scr, scr2,
                mult_eng="dve", red="act")
        _ip(nc, partials, _c_col(k), wk, dds, scr, scr2,
            mult_eng="gpsimd", red="dve")

        # Gram-Schmidt -> v_{k+1}; ||u||^2 = O_kk - sum c_j^2 (no 2nd pass)
        if k < RANK - 1:
            kk = k + 1
            for j in range(kk):
                _ip(nc, partials, _gs_col(kk, j), wk, v_tiles[j], scr, scr2,
                    mult_eng="dve", red="act")
            # one broadcast for O_kk + c_k + GS coefs: cols RBASE[k]+2k ..
            b0 = RBASE[k] + 2 * k
            nb = 4 + 2 * kk
            bc = _bcast(nc, ps_sm, ones, partials, b0, nb)
            s = scal.tile([HI, nb + 2 * kk + 6], F32, tag="gs_s")
            nc.scalar.copy(s[:, 0:nb], bc)
            x = nb
            coefs = s[:, x : x + kk]
            # pair-sum GS cols (offset 4 within block: after O_kk, c_k pairs)
            nc.vector.tensor_add(coefs, s[:, 4 : 4 + 2 * kk : 2],
                                 s[:, 5 : 4 + 2 * kk : 2])
            okk = s[:, x + kk : x + kk + 1]
            # ||u||^2 = (O_kk_hi + O_kk_lo) - sum_j coef_j^2
            u2 = s[:, x + kk + 1 : x + kk + 2]
            sq = s[:, x + kk + 2 : x + kk + 2 + kk]
            nc.vector.tensor_mul(sq, coefs, coefs)
            nc.vector.tensor_reduce(out=u2, in_=sq, axis=mybir.AxisListType.X,
                                    op=ALU.add)
            nc.vector.tensor_add(okk, s[:, 0:1], s[:, 1:2])
            u2b = s[:, x + 2 * kk + 2 : x + 2 * kk + 3]
            nc.vector.tensor_sub(u2b, okk, u2)
            sqr = s[:, x + 2 * kk + 3 : x + 2 * kk + 4]
            nc.scalar.sqrt(sqr, u2b)
            invn = s[:, x + 2 * kk + 4 : x + 2 * kk + 5]
            nc.vector.reciprocal(invn, sqr)

            u_hi = temps.tile([HI, N], F32, tag="u_hi")
            u_lo = temps.tile([LO, N], F32, tag="u_lo")
            for j in range(kk):
                ax_hi = temps.tile([HI, N], F32, tag="ax_hi")
                ax_lo = temps.tile([LO, N], F32, tag="ax_lo")
                nc.scalar.mul(ax_hi, v_tiles[j][0], coefs[:, j : j + 1])
                nc.scalar.mul(ax_lo, v_tiles[j][1], coefs[:LO, j : j + 1])
                src = wk if j == 0 else (u_hi, u_lo)
                nc.gpsimd.tensor_sub(u_hi, src[0], ax_hi)
                nc.gpsimd.tensor_sub(u_lo, src[1], ax_lo)
            vn_hi = persist.tile([HI, N], F32, tag=f"v{kk}_hi")
            vn_lo = persist.tile([LO, N], F32, tag=f"v{kk}_lo")
            nc.scalar.mul(vn_hi, u_hi, invn)
            nc.scalar.mul(vn_lo, u_lo, invn[:LO, :])
            v_tiles.append((vn_hi, vn_lo))

    return {"partials": partials, "v": v_tiles}


def _solve_sym4(nc, g, s):
    """Batched symmetric 4x4 solve on [BLK,1] column APs.

    g: [BLK, 14] tile, cols 0..9 = O (00,10,11,20,21,22,30,31,32,33),
    cols 10..13 = rhs c.  s: [BLK, 16] scratch.  Returns y col APs (in s cols 0..3).
    Mirrors _solve_sym4_np below; keep in sync.
    """
    def col(t, i):
        return t[:, i : i + 1]

    ox = [GIX[q] for q in ("O00", "O10", "O11", "O20", "O21", "O22",
                           "O30", "O31", "O32", "O33")]
    a, bb, e, c, f, h, d, gg, i_, jj = (col(g, i) for i in ox)
    r0, r1, r2, r3 = (col(g, GIX[f"c{i}"]) for i in range(4))
    p0, p1, p2, p3 = (col(s, 4 + i) for i in range(4))
    l1, l2, l3 = (col(s, 8 + i) for i in range(3))
    t0, t1 = col(s, 11), col(s, 12)
    y0, y1, y2, y3 = (col(s, i) for i in range(4))

    mul = nc.vector.tensor_mul
    sub = nc.vector.tensor_sub
    rec = nc.vector.reciprocal

    def upd(x, l, src):  # x -= l*src
        mul(t0, l, src)
        sub(x, x, t0)

    rec(p0, a)
    mul(l1, bb, p0); mul(l2, c, p0); mul(l3, d, p0)
    upd(e, l1, bb); upd(f, l2, bb); upd(gg, l3, bb)
    upd(h, l2, c); upd(i_, l3, c); upd(jj, l3, d)
    upd(r1, l1, r0); upd(r2, l2, r0); upd(r3, l3, r0)

    rec(p1, e)
    mul(l2, f, p1); mul(l3, gg, p1)
    upd(h, l2, f); upd(i_, l3, f); upd(jj, l3, gg)
    upd(r2, l2, r1); upd(r3, l3, r1)

    rec(p2, h)
    mul(l3, i_, p2)
    upd(jj, l3, i_); upd(r3, l3, r2)

    rec(p3, jj)
    mul(y3, r3, p3)
    # back-substitution
    upd(r2, i_, y3); mul(y2, r2, p2)
    upd(r1, f, y2); upd(r1, gg, y3); mul(y1, r1, p1)
    upd(r0, bb, y1); upd(r0, c, y2); upd(r0, d, y3); mul(y0, r0, p0)
    return [y0, y1, y2, y3]


def _solve_sym4_np(G):
    """NumPy mirror of _solve_sym4 for verification. G: [n, 14] -> y [n, 4]."""
    G = G.copy()
    cols = [G[:, i : i + 1] for i in range(14)]
    a, bb, e, c, f, h, d, gg, i_, jj = cols[:10]
    r0, r1, r2, r3 = cols[10:]
    p0 = 1.0 / a
    l1, l2, l3 = bb * p0, c * p0, d * p0
    e = e - l1 * bb; f = f - l2 * bb; gg = gg - l3 * bb
    h = h - l2 * c; i_ = i_ - l3 * c; jj = jj - l3 * d
    r1 = r1 - l1 * r0; r2 = r2 - l2 * r0; r3 = r3 - l3 * r0
    p1 = 1.0 / e
    l2, l3 = f * p1, gg * p1
    h = h - l2 * f; i_ = i_ - l3 * f; jj = jj - l3 * gg
    r2 = r2 - l2 * r1; r3 = r3 - l3 * r1
    p2 = 1.0 / h
    l3 = i_ * p2
    jj = jj - l3 * i_; r3 = r3 - l3 * r2
    p3 = 1.0 / jj
    y3 = r3 * p3
    r2 = r2 - i_ * y3; y2 = r2 * p2
    r1 = r1 - f * y2; r1 = r1 - gg * y3; y1 = r1 * p1
    r0 = r0 - bb * y1; r0 = r0 - c * y2; r0 = r0 - d * y3; y0 = r0 * p0
    return np.concatenate([y0, y1, y2, y3], axis=1)


def _block_tail(nc, tc, OUT, b, mols, blk_state, consts, fast, temps, scal, blkp,
                ps_g, ps_sm, ones, sel, id8):
    # gather each mol's 14 O/c sums into [BLK, 14] via selector matmuls
    gath = ps_g.tile([BLK, 40], F32, tag="gath")
    for j, st in enumerate(blk_state):
        nc.tensor.matmul(gath, lhsT=sel[:, LO - 1 - j : LO - 1 - j + BLK],
                         rhs=st["partials"][:, 2:42],
                         start=(j == 0), stop=(j == len(blk_state) - 1))
    g_pair = blkp.tile([BLK, 40], F32, tag="g_pair")
    nc.scalar.copy(g_pair, gath)
    g_sb = blkp.tile([BLK, 20], F32, tag="g_sb")
    nc.vector.tensor_add(g_sb, g_pair[:, 0:40:2], g_pair[:, 1:40:2])
    s_sb = blkp.tile([BLK, 16], F32, tag="s_sb")
    ys = _solve_sym4(nc, g_sb, s_sb)
    y_sb = blkp.tile([BLK, RANK], F32, tag="y_sb")
    for i in range(RANK):
        nc.vector.tensor_copy(y_sb[:, i : i + 1], ys[i])

    for j, (m, st) in enumerate(zip(mols, blk_state)):
        ymask = scal.tile([BLK, RANK], F32, tag="ymask")
        nc.vector.tensor_scalar(out=ymask, in0=y_sb, scalar1=id8[:, j : j + 1],
                                scalar2=None, op0=ALU.mult)
        ybc = ps_sm.tile([HI, RANK], F32, tag="bc")
        nc.tensor.matmul(ybc, lhsT=ones[0:BLK, :], rhs=ymask, start=True, stop=True)
        yb = scal.tile([HI, RANK], F32, tag="yb")
        nc.scalar.copy(yb, ybc)

        acc_hi = fast.tile([HI, N], F32, tag="acc_hi")
        acc_lo = fast.tile([LO, N], F32, tag="acc_lo")
        v = st["v"]
        nc.vector.tensor_scalar(out=acc_hi, in0=v[0][0], scalar1=yb[:, 0:1],
                                scalar2=-1.0, op0=ALU.mult, op1=ALU.mult)
        nc.vector.tensor_scalar(out=acc_lo, in0=v[0][1], scalar1=yb[:LO, 0:1],
                                scalar2=-1.0, op0=ALU.mult, op1=ALU.mult)
        for i in range(1, RANK):
            ax_hi = temps.tile([HI, N], F32, tag="ax_hi")
            ax_lo = temps.tile([LO, N], F32, tag="ax_lo")
            nc.vector.tensor_scalar(out=ax_hi, in0=v[i][0], scalar1=yb[:, i : i + 1],
                                    scalar2=None, op0=ALU.mult)
            nc.vector.tensor_scalar(out=ax_lo, in0=v[i][1], scalar1=yb[:LO, i : i + 1],
                                    scalar2=None, op0=ALU.mult)
            nc.gpsimd.tensor_sub(acc_hi, acc_hi, ax_hi)
            nc.gpsimd.tensor_sub(acc_lo, acc_lo, ax_lo)
        nc.sync.dma_start(out=OUT[m, 0:HI, :], in_=acc_hi)
        nc.sync.dma_start(out=OUT[m, HI:N, :], in_=acc_lo)


_NC_CACHE = None


def _get_nc():
    global _NC_CACHE
    if _NC_CACHE is None:
        _NC_CACHE = build_core_kernel()
    return _NC_CACHE


def kernel(D, P, R, max_rank=4, _trace=False):
    D = np.ascontiguousarray(D, dtype=np.float32)
    P = np.ascontiguousarray(P, dtype=np.float32)
    R = np.ascontiguousarray(R, dtype=np.float32)
    nc = _get_nc()
    in_maps = []
    for i in range(NCORES):
        sl = slice(i * MPC, (i + 1) * MPC)
        in_maps.append({"D": D[sl], "P": P[sl], "Rm": R[sl]})
    res = run_bass_kernel_spmd(nc, in_maps, core_ids=list(range(NCORES)),
                               trace=_trace)
    out = np.concatenate([r["OUT"] for r in res.results], axis=0)
    if _trace:
        kernel.last_exec_time_ns = res.exec_time_ns
        kernel.last_trace = res.instructions_and_trace
    return out


if __name__ == "__main__":
    # quick solver self-check
    rng = np.random.default_rng(0)
    A = rng.standard_normal((5, 4, 4)).astype(np.float32)
    M = np.einsum("bij,bkj->bik", A, A) + 4 * np.eye(4, dtype=np.float32)
    cv = rng.standard_normal((5, 4)).astype(np.float32)
    G = np.zeros((5, 14), dtype=np.float32)
    order = [(0, 0), (1, 0), (1, 1), (2, 0), (2, 1), (2, 2), (3, 0), (3, 1), (3, 2), (3, 3)]
    for ix, (k, j) in enumerate(order):
        G[:, ix] = M[:, k, j]
    G[:, 10:] = cv
    y = _solve_sym4_np(G)
    yref = np.stack([np.linalg.solve(M[i], cv[i]) for i in range(5)])
    print("solver max err:", np.abs(y - yref).max())



# revision 7
# speedup vs baseline: 4.2351x; 4.2351x over previous
"""XL-BOMD rank-4 Krylov propagation (EnergyXL) on 8 TRN2 NeuronCores.

Data-parallel over molecules: 512 mols -> 64 per core, processed in
pairs.  Per molecule (N=192, rank=4) the reference computes

    out = -V (W^T W)^{-1} W^T dDS,   W = F(V) = R V R - V

over the Gram-Schmidt basis V of the Krylov space K_4(dDS).  The
output is invariant under ANY invertible change of basis of K_4
(W is linear in V), so we use the raw power iterates S_k = R^k dDS R^k
directly:

    S_0 = D - P;  S_k = R S_{k-1} R              (8 bf16 PE products/mol)
    g[s] = <S_a, S_b>  (a+b = s, s = 0..8)       (Gram is Hankel: 9 ips)
    O[I,J] = g[I+J+2] - 2 g[I+J+1] + g[I+J],  c[J] = g[J+1] - g[J]
    y = O^{-1} c   (batched 4x4 Gauss over 32-mol blocks)
    out = -sum_I y_I S_I                         (fused scale-add chain)

Layout per pair (A, B): hi tiles [128, 384] (A rows 0:128 in cols
0:192, B in 192:384), lo tiles [128, 192] (A rows 128:192 in
partitions 0:64, B in 64:128).  Matmuls run in bf16 (PSUM fp32).
Gram inner products: DVE/Pool elementwise multiply (bf16 2x) then a
one-hot selector matmul on the PE column-sums each product into a
per-molecule row of a PSUM accumulator; a per-level tensor_reduce
lands g directly in the [32, 9] solver layout.
"""

import sys

sys.path.insert(0, "/opt/trn_rl_repo")

import numpy as np

import concourse.bass as bass
import concourse.bacc as bacc
import concourse.tile as tile
from concourse import mybir
from concourse.bass_utils import run_bass_kernel_spmd

F32 = mybir.dt.float32
BF16 = mybir.dt.bfloat16
ALU = mybir.AluOpType
ACTF = mybir.ActivationFunctionType

NMOL, N, RANK = 512, 192, 4
NCORES = 8
MPC = NMOL // NCORES      # 64 molecules per core
NPAIR = MPC // 2          # 32 pairs
BLKP = 16                 # pairs per block (32 mols -> one batched solve)
NBLK = NPAIR // BLKP
HI, LO = 128, 64

# g[s] = <S_a, S_b> with a+b = s; level k (k=1..4) computes s = 2k-1, 2k.
G_PAIRS = {0: (0, 0), 1: (0, 1), 2: (1, 1), 3: (1, 2), 4: (2, 2),
           5: (2, 3), 6: (3, 3), 7: (3, 4), 8: (4, 4)}


def build_core_kernel():
    nc = bacc.Bacc(None, target_bir_lowering=False, enable_partition_id=False)
    D = nc.dram_tensor("D", [MPC, N, N], F32, kind="ExternalInput")
    P = nc.dram_tensor("P", [MPC, N, N], F32, kind="ExternalInput")
    R = nc.dram_tensor("Rm", [MPC, N, N], F32, kind="ExternalInput")
    OUT = nc.dram_tensor("OUT", [MPC, N, N], F32, kind="ExternalOutput")
    with tile.TileContext(nc) as tc:
        _body(nc, tc, D, P, R, OUT)
    nc.finalize()
    return nc


def _consts(nc, pool):
    c = {}
    # cb1: one-hot ones-column selector bank (col 31 = all-ones, bf16);
    # window cb1[:, 31-r : 63-r] routes a column-sum to PSUM row r.
    cb1 = pool.tile([HI, 63], BF16)
    nc.vector.memset(cb1, 0.0)
    nc.vector.memset(cb1[:, 31:32], 1.0)
    c["cb1"] = cb1
    # cb2: col 31 = upper-half ones, col 32 = lower-half ones; window at
    # row r sends partitions 0:64 to row r and 64:128 to row r+1.
    cb2 = pool.tile([HI, 64], BF16)
    nc.vector.memset(cb2, 0.0)
    nc.vector.memset(cb2[0:LO, 31:32], 1.0)
    nc.vector.memset(cb2[LO:HI, 32:33], 1.0)
    c["cb2"] = cb2
    ones = pool.tile([HI, HI], F32)
    nc.vector.memset(ones, 1.0)
    c["ones"] = ones
    # selp: even partitions -> ones in cols 0:64, odd -> ones in 64:128.
    idp = pool.tile([HI, 1], mybir.dt.int32)
    nc.gpsimd.iota(idp, pattern=[[0, 1]], base=0, channel_multiplier=1)
    podd_i = pool.tile([HI, 1], mybir.dt.int32)
    nc.vector.tensor_scalar(out=podd_i, in0=idp, scalar1=1, scalar2=None,
                            op0=ALU.bitwise_and)
    podd = pool.tile([HI, 1], F32)
    nc.vector.tensor_scalar(out=podd, in0=podd_i, scalar1=1.0, scalar2=None,
                            op0=ALU.mult)
    pevn = pool.tile([HI, 1], F32)
    nc.vector.tensor_scalar(out=pevn, in0=podd, scalar1=-1.0, scalar2=1.0,
                            op0=ALU.mult, op1=ALU.add)
    selp = pool.tile([HI, HI], F32)
    nc.vector.tensor_scalar(out=selp[:, 0:LO], in0=ones[:, 0:LO],
                            scalar1=pevn, scalar2=None, op0=ALU.mult)
    nc.vector.tensor_scalar(out=selp[:, LO:HI], in0=ones[:, 0:LO],
                            scalar1=podd, scalar2=None, op0=ALU.mult)
    c["selp"] = selp
    # mask32[c, 4m+I] = (c == m); mask2[c, 4j+I] = (c in {2j, 2j+1})
    nm = 2 * BLKP
    mi = pool.tile([nm, 4 * nm], mybir.dt.int32)
    nc.gpsimd.iota(mi, pattern=[[-1, nm], [0, 4]], base=0,
                   channel_multiplier=1)
    mask32 = pool.tile([nm, 4 * nm], F32)
    nc.vector.tensor_scalar(out=mask32, in0=mi, scalar1=0, scalar2=None,
                            op0=ALU.is_equal)
    c["mask32"] = mask32
    mj = pool.tile([nm, 4 * BLKP], mybir.dt.int32)
    nc.gpsimd.iota(mj, pattern=[[-2, BLKP], [0, 4]], base=0,
                   channel_multiplier=1)
    m20 = pool.tile([nm, 4 * BLKP], F32)
    nc.vector.tensor_scalar(out=m20, in0=mj, scalar1=0, scalar2=None,
                            op0=ALU.is_equal)
    m21 = pool.tile([nm, 4 * BLKP], F32)
    nc.vector.tensor_scalar(out=m21, in0=mj, scalar1=1, scalar2=None,
                            op0=ALU.is_equal)
    mask2 = pool.tile([nm, 4 * BLKP], F32)
    nc.vector.tensor_add(mask2, m20, m21)
    c["mask2"] = mask2
    return c


def _body(nc, tc, D, P, R, OUT):
    import contextlib

    ctx = contextlib.ExitStack()
    with ctx:
        cpool = ctx.enter_context(tc.tile_pool(name="consts", bufs=1))
        sp = ctx.enter_context(tc.tile_pool(name="sp", bufs=BLKP + 2))
        tmp = ctx.enter_context(tc.tile_pool(name="tmp", bufs=3))
        tail = ctx.enter_context(tc.tile_pool(name="tail", bufs=2))
        ps_big = ctx.enter_context(tc.tile_pool(name="ps_big", bufs=2,
                                                space="PSUM"))
        ps_lo = ctx.enter_context(tc.tile_pool(name="ps_lo", bufs=2,
                                               space="PSUM"))
        ps_g = ctx.enter_context(tc.tile_pool(name="ps_g", bufs=3,
                                              space="PSUM"))
        ps_tl = ctx.enter_context(tc.tile_pool(name="ps_tl", bufs=1,
                                               space="PSUM"))
        C = _consts(nc, cpool)

        for b in range(NBLK):
            pairs = list(range(b * BLKP, (b + 1) * BLKP))
            st = {}
            g_sb = tail.tile([2 * BLKP, 9], F32, name="g_sb", tag="g_sb")
            for q in pairs:
                st[q] = _load_prep(nc, D, P, R, q, sp, tmp)
            _gram_level(nc, st, pairs, [0], C, tmp, ps_g, g_sb)
            for k in range(1, RANK + 1):
                _level(nc, st, pairs, k, sp, tmp, ps_big, ps_lo)
                _gram_level(nc, st, pairs, [2 * k - 1, 2 * k], C, tmp, ps_g,
                            g_sb)
            ybc = _tail(nc, pairs, C, tail, ps_tl, g_sb)
            for q in pairs:
                _combo(nc, st[q], q - b * BLKP, ybc, tmp)
            for q in pairs:
                _store(nc, OUT, q, st[q])


def _load_prep(nc, D, P, R, q, sp, tmp):
    """DMA loads, dDS = D - P (bf16), R cast for one pair."""
    mA, mB = 2 * q, 2 * q + 1
    stg = {}
    for nm, T in (("d", D), ("p", P), ("r", R)):
        sh = tmp.tile([HI, 2 * N], F32, name=f"{nm}sh", tag="stgh", bufs=6)
        sl = tmp.tile([HI, N], F32, name=f"{nm}sl", tag="stgl", bufs=6)
        nc.sync.dma_start(out=sh[:, 0:N], in_=T[mA, 0:HI, :])
        nc.sync.dma_start(out=sh[:, N:2 * N], in_=T[mB, 0:HI, :])
        nc.sync.dma_start(out=sl, in_=T[mA:mA + 2, HI:N, :])
        stg[nm] = (sh, sl)

    s0h = sp.tile([HI, 2 * N], BF16, tag="s0h")
    s0l = sp.tile([HI, N], BF16, tag="s0l")
    nc.vector.tensor_sub(s0h, stg["d"][0], stg["p"][0])
    nc.gpsimd.tensor_sub(s0l, stg["d"][1], stg["p"][1])
    rh = sp.tile([HI, 2 * N], BF16, tag="rh")
    rl = sp.tile([HI, N], BF16, tag="rl")
    nc.scalar.copy(rh, stg["r"][0])
    nc.gpsimd.tensor_copy(rl, stg["r"][1])
    return {"sh": [s0h], "sl": [s0l], "rh": rh, "rl": rl}


def _mm_pair(nc, ps_big, ps_lo, lhs_hi, lhs_lo, rhs_hi, rhs_lo, tagp):
    """One 192x192 @ 192x192 product for both pair mols -> PSUM pair tiles.

    out[p,f] = sum_c lhs[c,p] rhs[c,f] per molecule; lhs must be symmetric
    (we pass S or R directly as lhsT).
    """
    ph = ps_big.tile([HI, 2 * N], F32, name=f"ph_{tagp}", tag="pbig")
    pl = ps_lo.tile([HI, N], F32, name=f"pl_{tagp}", tag="plo")
    for m, c0, p0 in ((0, 0, 0), (1, N, LO)):  # mol A, mol B
        hi_c = lhs_hi[:, c0:c0 + HI]          # lhs cols 0:128 (out rows hi)
        hi_cl = lhs_hi[:, c0 + HI:c0 + N]     # lhs cols 128:192 (out rows lo)
        lo_c = lhs_lo[p0:p0 + LO, 0:HI]
        lo_cl = lhs_lo[p0:p0 + LO, HI:N]
        rhi = rhs_hi[:, c0:c0 + N]
        rlo = rhs_lo[p0:p0 + LO, :]
        nc.tensor.matmul(ph[:, c0:c0 + N], lhsT=hi_c, rhs=rhi,
                         start=True, stop=False)
        nc.tensor.matmul(ph[:, c0:c0 + N], lhsT=lo_c, rhs=rlo,
                         start=False, stop=True)
        nc.tensor.matmul(pl[p0:p0 + LO, :], lhsT=hi_cl, rhs=rhi,
                         start=True, stop=False)
        nc.tensor.matmul(pl[p0:p0 + LO, :], lhsT=lo_cl, rhs=rlo,
                         start=False, stop=True)
    return ph, pl


def _drain(nc, eng, out, in_):
    if eng == "dve":
        nc.vector.tensor_copy(out, in_)
    else:
        nc.scalar.copy(out, in_)


# engine schedule for the per-level PSUM drains (dve/act only: gpsimd
# cannot touch PSUM)
T_BIG_ENG = ["act", "act", "act", "act"]
S_BIG_ENG = ["act", "act", "act", "act"]
T_SM_ENG = ["dve", "act", "dve", "act"]
S_SM_ENG = ["act", "dve", "act", "dve"]


def _level(nc, st, pairs, k, sp, tmp, ps_big, ps_lo):
    """Level k: T = S_{k-1} R then S_k = R T for every pair."""
    tps = {}
    for q in pairs:
        s = st[q]
        tps[q] = _mm_pair(nc, ps_big, ps_lo, s["sh"][k - 1], s["sl"][k - 1],
                          s["rh"], s["rl"], f"t{k}_{q}")
    tts = {}
    for q in pairs:
        th = tmp.tile([HI, 2 * N], BF16, name=f"th{k}_{q}", tag="th",
                      bufs=BLKP + 2)
        tl = tmp.tile([HI, N], BF16, name=f"tl{k}_{q}", tag="tl",
                      bufs=BLKP + 2)
        _drain(nc, T_BIG_ENG[k - 1], th, tps[q][0])
        _drain(nc, T_SM_ENG[k - 1], tl, tps[q][1])
        tts[q] = (th, tl)
    sps = {}
    for q in pairs:
        s = st[q]
        sps[q] = _mm_pair(nc, ps_big, ps_lo, s["rh"], s["rl"],
                          tts[q][0], tts[q][1], f"s{k}_{q}")
    for q in pairs:
        s = st[q]
        skh = sp.tile([HI, 2 * N], BF16, name=f"s{k}h", tag=f"s{k}h",
                      bufs=BLKP + 2)
        skl = sp.tile([HI, N], BF16, name=f"s{k}l", tag=f"s{k}l",
                      bufs=BLKP + 2)
        _drain(nc, S_BIG_ENG[k - 1], skh, sps[q][0])
        _drain(nc, S_SM_ENG[k - 1], skl, sps[q][1])
        s["sh"].append(skh)
        s["sl"].append(skl)


# lo-tile Z-multiply engine per g-index (pool relieves DVE on some)
LO_MUL_POOL = {0, 2, 4, 6, 8}


def _gram_level(nc, st, pairs, svals, C, tmp, ps_g, g_sb):
    """g[s] = <S_a, S_b> for each s in svals, all pairs.

    Elementwise Z = S_a * S_b (DVE/Pool bf16), then one-hot selector
    matmuls column-sum Z into per-molecule rows of a [32, 192] PSUM
    accumulator; a final tensor_reduce writes g_sb[:, s].
    """
    cb1, cb2 = C["cb1"], C["cb2"]
    for s in svals:
        a, bb = G_PAIRS[s]
        gp = ps_g.tile([2 * BLKP, N], F32, name=f"gps{s}", tag="gps")
        nmm = 3 * len(pairs)
        i = 0
        for j, q in enumerate(pairs):
            stq = st[q]
            ah, bh = stq["sh"][a], stq["sh"][bb]
            al, bl = stq["sl"][a], stq["sl"][bb]
            zh = tmp.tile([HI, 2 * N], BF16, name="zh", tag="zh", bufs=6)
            zl = tmp.tile([HI, N], BF16, name="zl", tag="zl", bufs=6)
            nc.vector.tensor_mul(zh, ah, bh)
            if s in LO_MUL_POOL:
                nc.gpsimd.tensor_mul(zl, al, bl)
            else:
                nc.vector.tensor_mul(zl, al, bl)
            rA = 2 * j
            for lhsT, rhs in (
                (cb1[:, 31 - rA:63 - rA], zh[:, 0:N]),
                (cb1[:, 30 - rA:62 - rA], zh[:, N:2 * N]),
                (cb2[:, 31 - rA:63 - rA], zl),
            ):
                nc.tensor.matmul(gp[:, :], lhsT=lhsT, rhs=rhs,
                                 start=(i == 0), stop=(i == nmm - 1))
                i += 1
        nc.vector.tensor_reduce(out=g_sb[:, s:s + 1], in_=gp,
                                axis=mybir.AxisListType.X, op=ALU.add)


def _tail(nc, pairs, C, tail, ps_tl, g_sb):
    """Batched 4x4 solve from g, then broadcast -y to [128, *] columns."""
    nm = 2 * BLKP  # 32 molecules
    g = g_sb
    # Hankel assembly: h[s] = g[s] - 2 g[s+1] + g[s+2]; rhs c = diff(g)
    hs = tail.tile([nm, 7], F32, tag="hs")
    hm = tail.tile([nm, 7], F32, tag="hm")
    h = tail.tile([nm, 7], F32, tag="h")
    nc.vector.tensor_add(hs, g[:, 0:7], g[:, 2:9])
    nc.vector.tensor_scalar(out=hm, in0=g[:, 1:8], scalar1=-2.0, scalar2=None,
                            op0=ALU.mult)
    nc.vector.tensor_add(h, hs, hm)
    sv = tail.tile([nm, 14], F32, tag="sv")
    nc.vector.tensor_copy(sv[:, 0:4], h[:, 0:4])
    nc.vector.tensor_copy(sv[:, 4:7], h[:, 2:5])
    nc.vector.tensor_copy(sv[:, 7:9], h[:, 4:6])
    nc.vector.tensor_copy(sv[:, 9:10], h[:, 6:7])
    nc.vector.tensor_sub(sv[:, 10:14], g[:, 1:5], g[:, 0:4])

    ysb = _solve(nc, sv, tail, nm)
    ysn = tail.tile([nm, 4], F32, tag="ysn")
    nc.vector.tensor_scalar(out=ysn, in0=ysb, scalar1=-1.0, scalar2=None,
                            op0=ALU.mult)

    # broadcast -y to all partitions: cols 4m:(4m+4) per mol; cols
    # 128+4q:(128+4q+4) carry the packed-lo per-partition-half values.
    ones, selp = C["ones"], C["selp"]
    ysn_b = ysn.unsqueeze(1)
    yp = tail.tile([nm, 4 * nm], F32, tag="yp")
    nc.vector.tensor_mul(
        yp.rearrange("p (m i) -> p m i", i=4),
        C["mask32"].rearrange("p (m i) -> p m i", i=4),
        ysn_b.broadcast_to([nm, nm, 4]))
    yq = tail.tile([nm, 4 * BLKP], F32, tag="yq")
    nc.vector.tensor_mul(
        yq.rearrange("p (m i) -> p m i", i=4),
        C["mask2"].rearrange("p (m i) -> p m i", i=4),
        ysn_b.broadcast_to([nm, BLKP, 4]))
    ybp = ps_tl.tile([HI, N], F32, tag="ybp")
    nc.tensor.matmul(ybp[:, 0:4 * nm], lhsT=ones[0:nm, 0:HI], rhs=yp,
                     start=True, stop=True)
    nc.tensor.matmul(ybp[:, HI:HI + 4 * BLKP], lhsT=selp[0:nm, 0:HI],
                     rhs=yq, start=True, stop=True)
    ybc = tail.tile([HI, N], F32, tag="ybc")
    nc.scalar.copy(ybc, ybp)
    return ybc


def _solve(nc, sv, tail, nm):
    """Batched symmetric 4x4 Gauss elimination on [nm,1] column APs.

    sv cols: 0:a 1:b 2:c 3:d | 4:e 5:f 6:g | 7:h 8:i | 9:j | 10..13 r0..r3.
    Mirrors solve_batched_np (validated offline).
    """
    pp = tail.tile([nm, 4], F32, tag="pp")
    l3 = tail.tile([nm, 3], F32, tag="l3")
    tt = tail.tile([nm, 3], F32, tag="tt")
    ysb = tail.tile([nm, 4], F32, tag="ysb")

    ts = nc.vector.tensor_scalar
    sub = nc.vector.tensor_sub
    rec = nc.vector.reciprocal

    def upd(dst, src, scal, w=1):
        ts(out=tt[:, 0:w], in0=src, scalar1=scal, scalar2=None, op0=ALU.mult)
        sub(dst, dst, tt[:, 0:w])

    rec(pp[:, 0:1], sv[:, 0:1])
    ts(out=l3, in0=sv[:, 1:4], scalar1=pp[:, 0:1], scalar2=None, op0=ALU.mult)
    upd(sv[:, 4:7], l3, sv[:, 1:2], 3)          # (e,f,g) -= l*b
    upd(sv[:, 7:9], l3[:, 1:3], sv[:, 2:3], 2)  # (h,i) -= (l2,l3)*c
    upd(sv[:, 9:10], l3[:, 2:3], sv[:, 3:4])    # j -= l3*d
    upd(sv[:, 11:14], l3, sv[:, 10:11], 3)      # (r1,r2,r3) -= l*r0
    rec(pp[:, 1:2], sv[:, 4:5])
    ts(out=l3[:, 1:3], in0=sv[:, 5:7], scalar1=pp[:, 1:2], scalar2=None,
       op0=ALU.mult)
    upd(sv[:, 7:9], l3[:, 1:3], sv[:, 5:6], 2)
    upd(sv[:, 9:10], l3[:, 2:3], sv[:, 6:7])
    upd(sv[:, 12:14], l3[:, 1:3], sv[:, 11:12], 2)
    rec(pp[:, 2:3], sv[:, 7:8])
    ts(out=l3[:, 2:3], in0=sv[:, 8:9], scalar1=pp[:, 2:3], scalar2=None,
       op0=ALU.mult)
    upd(sv[:, 9:10], l3[:, 2:3], sv[:, 8:9])
    upd(sv[:, 13:14], l3[:, 2:3], sv[:, 12:13])
    rec(pp[:, 3:4], sv[:, 9:10])
    ts(out=ysb[:, 3:4], in0=sv[:, 13:14], scalar1=pp[:, 3:4], scalar2=None,
       op0=ALU.mult)
    upd(sv[:, 12:13], sv[:, 8:9], ysb[:, 3:4])
    ts(out=ysb[:, 2:3], in0=sv[:, 12:13], scalar1=pp[:, 2:3], scalar2=None,
       op0=ALU.mult)
    upd(sv[:, 11:12], sv[:, 5:6], ysb[:, 2:3])
    upd(sv[:, 11:12], sv[:, 6:7], ysb[:, 3:4])
    ts(out=ysb[:, 1:2], in0=sv[:, 11:12], scalar1=pp[:, 1:2], scalar2=None,
       op0=ALU.mult)
    upd(sv[:, 10:11], sv[:, 1:2], ysb[:, 1:2])
    upd(sv[:, 10:11], sv[:, 2:3], ysb[:, 2:3])
    upd(sv[:, 10:11], sv[:, 3:4], ysb[:, 3:4])
    ts(out=ysb[:, 0:1], in0=sv[:, 10:11], scalar1=pp[:, 0:1], scalar2=None,
       op0=ALU.mult)
    return ysb


def _combo(nc, stq, j, ybc, tmp):
    """acc = sum_I (-y_I) S_I via ts-multiplies + add chains."""
    mA, mB = 2 * j, 2 * j + 1
    ah = tmp.tile([HI, 2 * N], F32, name="acch", tag="acch", bufs=4)
    al = tmp.tile([HI, N], F32, name="accl", tag="accl", bufs=4)
    for m, c0 in ((mA, 0), (mB, N)):
        u = []
        for I in range(RANK):
            ut = tmp.tile([HI, N], BF16, name="cu", tag="cu", bufs=6)
            nc.vector.tensor_scalar(out=ut, in0=stq["sh"][I][:, c0:c0 + N],
                                    scalar1=ybc[:, 4 * m + I:4 * m + I + 1],
                                    scalar2=None, op0=ALU.mult)
            u.append(ut)
        w0 = tmp.tile([HI, N], BF16, name="cw0", tag="cw", bufs=4)
        w1 = tmp.tile([HI, N], BF16, name="cw1", tag="cw", bufs=4)
        nc.vector.tensor_add(w0, u[0], u[1])
        nc.vector.tensor_add(w1, u[2], u[3])
        nc.vector.tensor_add(ah[:, c0:c0 + N], w0, w1)
    u = []
    for I in range(RANK):
        ut = tmp.tile([HI, N], BF16, name="cul", tag="cu", bufs=6)
        nc.vector.tensor_scalar(out=ut, in0=stq["sl"][I],
                                scalar1=ybc[:, HI + 4 * j + I:HI + 4 * j + I + 1],
                                scalar2=None, op0=ALU.mult)
        u.append(ut)
    w0 = tmp.tile([HI, N], BF16, name="cwl0", tag="cw", bufs=4)
    w1 = tmp.tile([HI, N], BF16, name="cwl1", tag="cw", bufs=4)
    nc.gpsimd.tensor_add(w0, u[0], u[1])
    nc.gpsimd.tensor_add(w1, u[2], u[3])
    nc.vector.tensor_add(al, w0, w1)
    stq["acc"] = (ah, al)


def _store(nc, OUT, q, stq):
    mA, mB = 2 * q, 2 * q + 1
    ah, al = stq["acc"]
    nc.sync.dma_start(out=OUT[mA, 0:HI, :], in_=ah[:, 0:N])
    nc.sync.dma_start(out=OUT[mB, 0:HI, :], in_=ah[:, N:2 * N])
    nc.sync.dma_start(out=OUT[mA:mA + 2, HI:N, :], in_=al)


_NC_CACHE = None


def _get_nc():
    global _NC_CACHE
    if _NC_CACHE is None:
        _NC_CACHE = build_core_kernel()
    return _NC_CACHE


def kernel(D, P, R, max_rank=4, _trace=False):
    D = np.ascontiguousarray(D, dtype=np.float32)
    P = np.ascontiguousarray(P, dtype=np.float32)
    R = np.ascontiguousarray(R, dtype=np.float32)
    nc = _get_nc()
    in_maps = []
    for i in range(NCORES):
        sl = slice(i * MPC, (i + 1) * MPC)
        in_maps.append({"D": D[sl], "P": P[sl], "Rm": R[sl]})
    res = run_bass_kernel_spmd(nc, in_maps, core_ids=list(range(NCORES)),
                               trace=_trace)
    out = np.concatenate([r["OUT"] for r in res.results], axis=0)
    if _trace:
        kernel.last_exec_time_ns = res.exec_time_ns
        kernel.last_trace = res.instructions_and_trace
    return out


if __name__ == "__main__":
    import tempfile
    from concourse.bass_utils import compile_bass_kernel
    nc = build_core_kernel()
    print("build OK")
    if "--compile" in sys.argv:
        td = tempfile.mkdtemp()
        print("NEFF:", compile_bass_kernel(nc, td))


# revision 9
# speedup vs baseline: 4.2749x; 1.0094x over previous
"""XL-BOMD rank-4 Krylov propagation (EnergyXL) on 8 TRN2 NeuronCores.

Data-parallel over molecules: 512 mols -> 64 per core, processed in
pairs.  Per molecule (N=192, rank=4) the reference computes

    out = -V (W^T W)^{-1} W^T dDS,   W = F(V) = R V R - V

over the Gram-Schmidt basis V of the Krylov space K_4(dDS).  The
output is invariant under ANY invertible change of basis of K_4
(W is linear in V), so we use the raw power iterates S_k = R^k dDS R^k
directly:

    S_0 = D - P;  S_k = R S_{k-1} R              (8 bf16 PE products/mol)
    g[s] = <S_a, S_b>  (a+b = s, s = 0..8)       (Gram is Hankel: 9 ips)
    O[I,J] = g[I+J+2] - 2 g[I+J+1] + g[I+J],  c[J] = g[J+1] - g[J]
    y = O^{-1} c   (batched 4x4 Gauss over 32-mol blocks)
    out = -sum_I y_I S_I                         (fused scale-add chain)

Layout per pair (A, B): hi tiles [128, 384] (A rows 0:128 in cols
0:192, B in 192:384), lo tiles [128, 192] (A rows 128:192 in
partitions 0:64, B in 64:128).  Matmuls run in bf16 (PSUM fp32).
Gram inner products: DVE/Pool elementwise multiply (bf16 2x) then a
one-hot selector matmul on the PE column-sums each product into a
per-molecule row of a PSUM accumulator; a per-level tensor_reduce
lands g directly in the [32, 9] solver layout.
"""

import sys

sys.path.insert(0, "/opt/trn_rl_repo")

import numpy as np

import concourse.bass as bass
import concourse.bacc as bacc
import concourse.tile as tile
from concourse import mybir
from concourse.bass_utils import run_bass_kernel_spmd

F32 = mybir.dt.float32
BF16 = mybir.dt.bfloat16
ALU = mybir.AluOpType
ACTF = mybir.ActivationFunctionType

NMOL, N, RANK = 512, 192, 4
NCORES = 8
MPC = NMOL // NCORES      # 64 molecules per core
NPAIR = MPC // 2          # 32 pairs
BLKP = 16                 # pairs per block (32 mols -> one batched solve)
NBLK = NPAIR // BLKP
HI, LO = 128, 64

# g[s] = <S_a, S_b> with a+b = s; level k (k=1..4) computes s = 2k-1, 2k.
G_PAIRS = {0: (0, 0), 1: (0, 1), 2: (1, 1), 3: (1, 2), 4: (2, 2),
           5: (2, 3), 6: (3, 3), 7: (3, 4), 8: (4, 4)}


def build_core_kernel():
    nc = bacc.Bacc(None, target_bir_lowering=False, enable_partition_id=False)
    D = nc.dram_tensor("D", [MPC, N, N], F32, kind="ExternalInput")
    P = nc.dram_tensor("P", [MPC, N, N], F32, kind="ExternalInput")
    R = nc.dram_tensor("Rm", [MPC, N, N], F32, kind="ExternalInput")
    OUT = nc.dram_tensor("OUT", [MPC, N, N], F32, kind="ExternalOutput")
    with tile.TileContext(nc) as tc:
        _body(nc, tc, D, P, R, OUT)
    nc.finalize()
    return nc


def _consts(nc, pool):
    c = {}
    # cb1: one-hot ones-column selector bank (col 31 = all-ones, bf16);
    # window cb1[:, 31-r : 63-r] routes a column-sum to PSUM row r.
    cb1 = pool.tile([HI, 63], BF16)
    nc.vector.memset(cb1, 0.0)
    nc.vector.memset(cb1[:, 31:32], 1.0)
    c["cb1"] = cb1
    # cb2: col 31 = upper-half ones, col 32 = lower-half ones; window at
    # row r sends partitions 0:64 to row r and 64:128 to row r+1.
    cb2 = pool.tile([HI, 64], BF16)
    nc.vector.memset(cb2, 0.0)
    nc.vector.memset(cb2[0:LO, 31:32], 1.0)
    nc.vector.memset(cb2[LO:HI, 32:33], 1.0)
    c["cb2"] = cb2
    ones = pool.tile([HI, HI], F32)
    nc.vector.memset(ones, 1.0)
    c["ones"] = ones
    # selp: even partitions -> ones in cols 0:64, odd -> ones in 64:128.
    idp = pool.tile([HI, 1], mybir.dt.int32)
    nc.gpsimd.iota(idp, pattern=[[0, 1]], base=0, channel_multiplier=1)
    podd_i = pool.tile([HI, 1], mybir.dt.int32)
    nc.vector.tensor_scalar(out=podd_i, in0=idp, scalar1=1, scalar2=None,
                            op0=ALU.bitwise_and)
    podd = pool.tile([HI, 1], F32)
    nc.vector.tensor_scalar(out=podd, in0=podd_i, scalar1=1.0, scalar2=None,
                            op0=ALU.mult)
    pevn = pool.tile([HI, 1], F32)
    nc.vector.tensor_scalar(out=pevn, in0=podd, scalar1=-1.0, scalar2=1.0,
                            op0=ALU.mult, op1=ALU.add)
    selp = pool.tile([HI, HI], F32)
    nc.vector.tensor_scalar(out=selp[:, 0:LO], in0=ones[:, 0:LO],
                            scalar1=pevn, scalar2=None, op0=ALU.mult)
    nc.vector.tensor_scalar(out=selp[:, LO:HI], in0=ones[:, 0:LO],
                            scalar1=podd, scalar2=None, op0=ALU.mult)
    c["selp"] = selp
    # mask32[c, 4m+I] = (c == m); mask2[c, 4j+I] = (c in {2j, 2j+1})
    nm = 2 * BLKP
    mi = pool.tile([nm, 4 * nm], mybir.dt.int32)
    nc.gpsimd.iota(mi, pattern=[[-1, nm], [0, 4]], base=0,
                   channel_multiplier=1)
    mask32 = pool.tile([nm, 4 * nm], F32)
    nc.vector.tensor_scalar(out=mask32, in0=mi, scalar1=0, scalar2=None,
                            op0=ALU.is_equal)
    c["mask32"] = mask32
    mj = pool.tile([nm, 4 * BLKP], mybir.dt.int32)
    nc.gpsimd.iota(mj, pattern=[[-2, BLKP], [0, 4]], base=0,
                   channel_multiplier=1)
    m20 = pool.tile([nm, 4 * BLKP], F32)
    nc.vector.tensor_scalar(out=m20, in0=mj, scalar1=0, scalar2=None,
                            op0=ALU.is_equal)
    m21 = pool.tile([nm, 4 * BLKP], F32)
    nc.vector.tensor_scalar(out=m21, in0=mj, scalar1=1, scalar2=None,
                            op0=ALU.is_equal)
    mask2 = pool.tile([nm, 4 * BLKP], F32)
    nc.vector.tensor_add(mask2, m20, m21)
    c["mask2"] = mask2
    return c


def _body(nc, tc, D, P, R, OUT):
    import contextlib

    ctx = contextlib.ExitStack()
    with ctx:
        cpool = ctx.enter_context(tc.tile_pool(name="consts", bufs=1))
        sp = ctx.enter_context(tc.tile_pool(name="sp", bufs=BLKP + 2))
        tmp = ctx.enter_context(tc.tile_pool(name="tmp", bufs=3))
        tail = ctx.enter_context(tc.tile_pool(name="tail", bufs=2))
        ps_big = ctx.enter_context(tc.tile_pool(name="ps_big", bufs=3,
                                                space="PSUM"))
        ps_lo = ctx.enter_context(tc.tile_pool(name="ps_lo", bufs=3,
                                               space="PSUM"))
        ps_g = ctx.enter_context(tc.tile_pool(name="ps_g", bufs=1,
                                              space="PSUM"))
        ps_tl = ctx.enter_context(tc.tile_pool(name="ps_tl", bufs=1,
                                               space="PSUM"))
        C = _consts(nc, cpool)

        for b in range(NBLK):
            pairs = list(range(b * BLKP, (b + 1) * BLKP))
            st = {}
            g_sb = tail.tile([2 * BLKP, 9], F32, name="g_sb", tag="g_sb")
            for q in pairs:
                st[q] = _load_prep(nc, D, P, R, q, sp, tmp)
            _gram_level(nc, st, pairs, [0], C, tmp, ps_g, g_sb)
            for k in range(1, RANK + 1):
                _level(nc, st, pairs, k, sp, tmp, ps_big, ps_lo)
                _gram_level(nc, st, pairs, [2 * k - 1, 2 * k], C, tmp, ps_g,
                            g_sb)
            ybc = _tail(nc, pairs, C, tail, ps_tl, g_sb)
            for q in pairs:
                _combo(nc, st[q], q - b * BLKP, ybc, tmp)
            for q in pairs:
                _store(nc, OUT, q, st[q])


def _load_prep(nc, D, P, R, q, sp, tmp):
    """DMA loads, dDS = D - P (bf16), R cast for one pair."""
    mA, mB = 2 * q, 2 * q + 1
    stg = {}
    for nm, T in (("d", D), ("p", P), ("r", R)):
        sh = tmp.tile([HI, 2 * N], F32, name=f"{nm}sh", tag="stgh", bufs=6)
        sl = tmp.tile([HI, N], F32, name=f"{nm}sl", tag="stgl", bufs=6)
        nc.sync.dma_start(out=sh[:, 0:N], in_=T[mA, 0:HI, :])
        nc.sync.dma_start(out=sh[:, N:2 * N], in_=T[mB, 0:HI, :])
        nc.sync.dma_start(out=sl, in_=T[mA:mA + 2, HI:N, :])
        stg[nm] = (sh, sl)

    s0h = sp.tile([HI, 2 * N], BF16, tag="s0h")
    s0l = sp.tile([HI, N], BF16, tag="s0l")
    nc.vector.tensor_sub(s0h, stg["d"][0], stg["p"][0])
    nc.gpsimd.tensor_sub(s0l, stg["d"][1], stg["p"][1])
    rh = sp.tile([HI, 2 * N], BF16, tag="rh")
    rl = sp.tile([HI, N], BF16, tag="rl")
    nc.scalar.copy(rh, stg["r"][0])
    nc.gpsimd.tensor_copy(rl, stg["r"][1])
    return {"sh": [s0h], "sl": [s0l], "rh": rh, "rl": rl}


def _mm_pair(nc, ps_big, ps_lo, lhs_hi, lhs_lo, rhs_hi, rhs_lo, tagp):
    """One 192x192 @ 192x192 product for both pair mols -> PSUM pair tiles.

    out[p,f] = sum_c lhs[c,p] rhs[c,f] per molecule; lhs must be symmetric
    (we pass S or R directly as lhsT).
    """
    ph = ps_big.tile([HI, 2 * N], F32, name=f"ph_{tagp}", tag="pbig")
    pl = ps_lo.tile([HI, N], F32, name=f"pl_{tagp}", tag="plo")
    for m, c0, p0 in ((0, 0, 0), (1, N, LO)):  # mol A, mol B
        hi_c = lhs_hi[:, c0:c0 + HI]          # lhs cols 0:128 (out rows hi)
        hi_cl = lhs_hi[:, c0 + HI:c0 + N]     # lhs cols 128:192 (out rows lo)
        lo_c = lhs_lo[p0:p0 + LO, 0:HI]
        lo_cl = lhs_lo[p0:p0 + LO, HI:N]
        rhi = rhs_hi[:, c0:c0 + N]
        rlo = rhs_lo[p0:p0 + LO, :]
        nc.tensor.matmul(ph[:, c0:c0 + N], lhsT=hi_c, rhs=rhi,
                         start=True, stop=False)
        nc.tensor.matmul(ph[:, c0:c0 + N], lhsT=lo_c, rhs=rlo,
                         start=False, stop=True)
        nc.tensor.matmul(pl[p0:p0 + LO, :], lhsT=hi_cl, rhs=rhi,
                         start=True, stop=False)
        nc.tensor.matmul(pl[p0:p0 + LO, :], lhsT=lo_cl, rhs=rlo,
                         start=False, stop=True)
    return ph, pl


def _drain(nc, eng, out, in_):
    if eng == "dve":
        nc.vector.tensor_copy(out, in_)
    else:
        nc.scalar.copy(out, in_)


# engine schedule for the per-level PSUM drains (dve/act only: gpsimd
# cannot touch PSUM)
T_BIG_ENG = ["act", "act", "act", "act"]
S_BIG_ENG = ["act", "act", "act", "act"]
T_SM_ENG = ["dve", "act", "dve", "act"]
S_SM_ENG = ["act", "dve", "act", "dve"]


def _level(nc, st, pairs, k, sp, tmp, ps_big, ps_lo):
    """Level k: T = S_{k-1} R then S_k = R T for every pair."""
    tps = {}
    for q in pairs:
        s = st[q]
        tps[q] = _mm_pair(nc, ps_big, ps_lo, s["sh"][k - 1], s["sl"][k - 1],
                          s["rh"], s["rl"], f"t{k}_{q}")
    tts = {}
    for q in pairs:
        th = tmp.tile([HI, 2 * N], BF16, name=f"th{k}_{q}", tag="th",
                      bufs=BLKP + 2)
        tl = tmp.tile([HI, N], BF16, name=f"tl{k}_{q}", tag="tl",
                      bufs=BLKP + 2)
        _drain(nc, T_BIG_ENG[k - 1], th, tps[q][0])
        _drain(nc, T_SM_ENG[k - 1], tl, tps[q][1])
        tts[q] = (th, tl)
    sps = {}
    for q in pairs:
        s = st[q]
        sps[q] = _mm_pair(nc, ps_big, ps_lo, s["rh"], s["rl"],
                          tts[q][0], tts[q][1], f"s{k}_{q}")
    for q in pairs:
        s = st[q]
        skh = sp.tile([HI, 2 * N], BF16, name=f"s{k}h", tag=f"s{k}h",
                      bufs=BLKP + 2)
        skl = sp.tile([HI, N], BF16, name=f"s{k}l", tag=f"s{k}l",
                      bufs=BLKP + 2)
        _drain(nc, S_BIG_ENG[k - 1], skh, sps[q][0])
        _drain(nc, S_SM_ENG[k - 1], skl, sps[q][1])
        s["sh"].append(skh)
        s["sl"].append(skl)


# lo-tile Z-multiply engine per g-index (pool relieves DVE on some)
LO_MUL_POOL = {0, 2, 4, 6, 8}


def _gram_level(nc, st, pairs, svals, C, tmp, ps_g, g_sb):
    """g[s] = <S_a, S_b> for each s in svals, all pairs.

    Elementwise Z = S_a * S_b (DVE/Pool bf16), then one-hot selector
    matmuls column-sum Z into per-molecule rows of a [32, 192] PSUM
    accumulator; a final tensor_reduce writes g_sb[:, s].
    """
    cb1, cb2 = C["cb1"], C["cb2"]
    for s in svals:
        a, bb = G_PAIRS[s]
        gp = ps_g.tile([2 * BLKP, N], F32, name=f"gps{s}", tag="gps")
        nmm = 3 * len(pairs)
        i = 0
        for j, q in enumerate(pairs):
            stq = st[q]
            ah, bh = stq["sh"][a], stq["sh"][bb]
            al, bl = stq["sl"][a], stq["sl"][bb]
            zh = tmp.tile([HI, 2 * N], BF16, name="zh", tag="zh", bufs=6)
            zl = tmp.tile([HI, N], BF16, name="zl", tag="zl", bufs=6)
            nc.vector.tensor_mul(zh, ah, bh)
            if s in LO_MUL_POOL:
                nc.gpsimd.tensor_mul(zl, al, bl)
            else:
                nc.vector.tensor_mul(zl, al, bl)
            rA = 2 * j
            for lhsT, rhs in (
                (cb1[:, 31 - rA:63 - rA], zh[:, 0:N]),
                (cb1[:, 30 - rA:62 - rA], zh[:, N:2 * N]),
                (cb2[:, 31 - rA:63 - rA], zl),
            ):
                nc.tensor.matmul(gp[:, :], lhsT=lhsT, rhs=rhs,
                                 start=(i == 0), stop=(i == nmm - 1))
                i += 1
        nc.vector.tensor_reduce(out=g_sb[:, s:s + 1], in_=gp,
                                axis=mybir.AxisListType.X, op=ALU.add)


def _tail(nc, pairs, C, tail, ps_tl, g_sb):
    """Batched 4x4 solve from g, then broadcast -y to [128, *] columns."""
    nm = 2 * BLKP  # 32 molecules
    g = g_sb
    # Hankel assembly: h[s] = g[s] - 2 g[s+1] + g[s+2]; rhs c = diff(g)
    hs = tail.tile([nm, 7], F32, tag="hs")
    hm = tail.tile([nm, 7], F32, tag="hm")
    h = tail.tile([nm, 7], F32, tag="h")
    nc.vector.tensor_add(hs, g[:, 0:7], g[:, 2:9])
    nc.vector.tensor_scalar(out=hm, in0=g[:, 1:8], scalar1=-2.0, scalar2=None,
                            op0=ALU.mult)
    nc.vector.tensor_add(h, hs, hm)
    sv = tail.tile([nm, 14], F32, tag="sv")
    nc.vector.tensor_copy(sv[:, 0:4], h[:, 0:4])
    nc.vector.tensor_copy(sv[:, 4:7], h[:, 2:5])
    nc.vector.tensor_copy(sv[:, 7:9], h[:, 4:6])
    nc.vector.tensor_copy(sv[:, 9:10], h[:, 6:7])
    nc.vector.tensor_sub(sv[:, 10:14], g[:, 1:5], g[:, 0:4])

    ysb = _solve(nc, sv, tail, nm)
    ysn = tail.tile([nm, 4], F32, tag="ysn")
    nc.vector.tensor_scalar(out=ysn, in0=ysb, scalar1=-1.0, scalar2=None,
                            op0=ALU.mult)

    # broadcast -y to all partitions: cols 4m:(4m+4) per mol; cols
    # 128+4q:(128+4q+4) carry the packed-lo per-partition-half values.
    ones, selp = C["ones"], C["selp"]
    ysn_b = ysn.unsqueeze(1)
    yp = tail.tile([nm, 4 * nm], F32, tag="yp")
    nc.vector.tensor_mul(
        yp.rearrange("p (m i) -> p m i", i=4),
        C["mask32"].rearrange("p (m i) -> p m i", i=4),
        ysn_b.broadcast_to([nm, nm, 4]))
    yq = tail.tile([nm, 4 * BLKP], F32, tag="yq")
    nc.vector.tensor_mul(
        yq.rearrange("p (m i) -> p m i", i=4),
        C["mask2"].rearrange("p (m i) -> p m i", i=4),
        ysn_b.broadcast_to([nm, BLKP, 4]))
    ybp = ps_tl.tile([HI, N], F32, tag="ybp")
    nc.tensor.matmul(ybp[:, 0:4 * nm], lhsT=ones[0:nm, 0:HI], rhs=yp,
                     start=True, stop=True)
    nc.tensor.matmul(ybp[:, HI:HI + 4 * BLKP], lhsT=selp[0:nm, 0:HI],
                     rhs=yq, start=True, stop=True)
    ybc = tail.tile([HI, N], F32, tag="ybc")
    nc.scalar.copy(ybc, ybp)
    return ybc


def _solve(nc, sv, tail, nm):
    """Batched symmetric 4x4 Gauss elimination on [nm,1] column APs.

    sv cols: 0:a 1:b 2:c 3:d | 4:e 5:f 6:g | 7:h 8:i | 9:j | 10..13 r0..r3.
    Mirrors solve_batched_np (validated offline).
    """
    pp = tail.tile([nm, 4], F32, tag="pp")
    l3 = tail.tile([nm, 3], F32, tag="l3")
    tt = tail.tile([nm, 3], F32, tag="tt")
    ysb = tail.tile([nm, 4], F32, tag="ysb")

    ts = nc.vector.tensor_scalar
    sub = nc.vector.tensor_sub
    rec = nc.vector.reciprocal

    def upd(dst, src, scal, w=1):
        ts(out=tt[:, 0:w], in0=src, scalar1=scal, scalar2=None, op0=ALU.mult)
        sub(dst, dst, tt[:, 0:w])

    rec(pp[:, 0:1], sv[:, 0:1])
    ts(out=l3, in0=sv[:, 1:4], scalar1=pp[:, 0:1], scalar2=None, op0=ALU.mult)
    upd(sv[:, 4:7], l3, sv[:, 1:2], 3)          # (e,f,g) -= l*b
    upd(sv[:, 7:9], l3[:, 1:3], sv[:, 2:3], 2)  # (h,i) -= (l2,l3)*c
    upd(sv[:, 9:10], l3[:, 2:3], sv[:, 3:4])    # j -= l3*d
    upd(sv[:, 11:14], l3, sv[:, 10:11], 3)      # (r1,r2,r3) -= l*r0
    rec(pp[:, 1:2], sv[:, 4:5])
    ts(out=l3[:, 1:3], in0=sv[:, 5:7], scalar1=pp[:, 1:2], scalar2=None,
       op0=ALU.mult)
    upd(sv[:, 7:9], l3[:, 1:3], sv[:, 5:6], 2)
    upd(sv[:, 9:10], l3[:, 2:3], sv[:, 6:7])
    upd(sv[:, 12:14], l3[:, 1:3], sv[:, 11:12], 2)
    rec(pp[:, 2:3], sv[:, 7:8])
    ts(out=l3[:, 2:3], in0=sv[:, 8:9], scalar1=pp[:, 2:3], scalar2=None,
       op0=ALU.mult)
    upd(sv[:, 9:10], l3[:, 2:3], sv[:, 8:9])
    upd(sv[:, 13:14], l3[:, 2:3], sv[:, 12:13])
    rec(pp[:, 3:4], sv[:, 9:10])
    ts(out=ysb[:, 3:4], in0=sv[:, 13:14], scalar1=pp[:, 3:4], scalar2=None,
       op0=ALU.mult)
    upd(sv[:, 12:13], sv[:, 8:9], ysb[:, 3:4])
    ts(out=ysb[:, 2:3], in0=sv[:, 12:13], scalar1=pp[:, 2:3], scalar2=None,
       op0=ALU.mult)
    upd(sv[:, 11:12], sv[:, 5:6], ysb[:, 2:3])
    upd(sv[:, 11:12], sv[:, 6:7], ysb[:, 3:4])
    ts(out=ysb[:, 1:2], in0=sv[:, 11:12], scalar1=pp[:, 1:2], scalar2=None,
       op0=ALU.mult)
    upd(sv[:, 10:11], sv[:, 1:2], ysb[:, 1:2])
    upd(sv[:, 10:11], sv[:, 2:3], ysb[:, 2:3])
    upd(sv[:, 10:11], sv[:, 3:4], ysb[:, 3:4])
    ts(out=ysb[:, 0:1], in0=sv[:, 10:11], scalar1=pp[:, 0:1], scalar2=None,
       op0=ALU.mult)
    return ysb


def _combo(nc, stq, j, ybc, tmp):
    """acc = sum_I (-y_I) S_I via ts-multiplies + add chains."""
    mA, mB = 2 * j, 2 * j + 1
    ah = tmp.tile([HI, 2 * N], F32, name="acch", tag="acch", bufs=4)
    al = tmp.tile([HI, N], F32, name="accl", tag="accl", bufs=4)
    for m, c0 in ((mA, 0), (mB, N)):
        u = []
        for I in range(RANK):
            ut = tmp.tile([HI, N], BF16, name="cu", tag="cu", bufs=6)
            nc.vector.tensor_scalar(out=ut, in0=stq["sh"][I][:, c0:c0 + N],
                                    scalar1=ybc[:, 4 * m + I:4 * m + I + 1],
                                    scalar2=None, op0=ALU.mult)
            u.append(ut)
        w0 = tmp.tile([HI, N], BF16, name="cw0", tag="cw", bufs=4)
        w1 = tmp.tile([HI, N], BF16, name="cw1", tag="cw", bufs=4)
        nc.vector.tensor_add(w0, u[0], u[1])
        nc.vector.tensor_add(w1, u[2], u[3])
        nc.vector.tensor_add(ah[:, c0:c0 + N], w0, w1)
    u = []
    for I in range(RANK):
        ut = tmp.tile([HI, N], BF16, name="cul", tag="cu", bufs=6)
        nc.vector.tensor_scalar(out=ut, in0=stq["sl"][I],
                                scalar1=ybc[:, HI + 4 * j + I:HI + 4 * j + I + 1],
                                scalar2=None, op0=ALU.mult)
        u.append(ut)
    w0 = tmp.tile([HI, N], BF16, name="cwl0", tag="cw", bufs=4)
    w1 = tmp.tile([HI, N], BF16, name="cwl1", tag="cw", bufs=4)
    nc.gpsimd.tensor_add(w0, u[0], u[1])
    nc.gpsimd.tensor_add(w1, u[2], u[3])
    nc.vector.tensor_add(al, w0, w1)
    stq["acc"] = (ah, al)


def _store(nc, OUT, q, stq):
    mA, mB = 2 * q, 2 * q + 1
    ah, al = stq["acc"]
    nc.sync.dma_start(out=OUT[mA, 0:HI, :], in_=ah[:, 0:N])
    nc.sync.dma_start(out=OUT[mB, 0:HI, :], in_=ah[:, N:2 * N])
    nc.sync.dma_start(out=OUT[mA:mA + 2, HI:N, :], in_=al)


_NC_CACHE = None


def _get_nc():
    global _NC_CACHE
    if _NC_CACHE is None:
        _NC_CACHE = build_core_kernel()
    return _NC_CACHE


def kernel(D, P, R, max_rank=4, _trace=False):
    D = np.ascontiguousarray(D, dtype=np.float32)
    P = np.ascontiguousarray(P, dtype=np.float32)
    R = np.ascontiguousarray(R, dtype=np.float32)
    nc = _get_nc()
    in_maps = []
    for i in range(NCORES):
        sl = slice(i * MPC, (i + 1) * MPC)
        in_maps.append({"D": D[sl], "P": P[sl], "Rm": R[sl]})
    res = run_bass_kernel_spmd(nc, in_maps, core_ids=list(range(NCORES)),
                               trace=_trace)
    out = np.concatenate([r["OUT"] for r in res.results], axis=0)
    if _trace:
        kernel.last_exec_time_ns = res.exec_time_ns
        kernel.last_trace = res.instructions_and_trace
    return out


if __name__ == "__main__":
    import tempfile
    from concourse.bass_utils import compile_bass_kernel
    nc = build_core_kernel()
    print("build OK")
    if "--compile" in sys.argv:
        td = tempfile.mkdtemp()
        print("NEFF:", compile_bass_kernel(nc, td))


# revision 16
# speedup vs baseline: 4.3291x; 1.0127x over previous
"""XL-BOMD rank-4 Krylov propagation (EnergyXL) on 8 TRN2 NeuronCores.

Data-parallel over molecules: 512 mols -> 64 per core, processed in
pairs.  Per molecule (N=192, rank=4) the reference computes

    out = -V (W^T W)^{-1} W^T dDS,   W = F(V) = R V R - V

over the Gram-Schmidt basis V of the Krylov space K_4(dDS).  The
output is invariant under ANY invertible change of basis of K_4
(W is linear in V), so we use the raw power iterates S_k = R^k dDS R^k
directly:

    S_0 = D - P;  S_k = R S_{k-1} R              (8 bf16 PE products/mol)
    g[s] = <S_a, S_b>  (a+b = s, s = 0..8)       (Gram is Hankel: 9 ips)
    O[I,J] = g[I+J+2] - 2 g[I+J+1] + g[I+J],  c[J] = g[J+1] - g[J]
    y = O^{-1} c   (batched 4x4 Gauss over 32-mol blocks)
    out = -sum_I y_I S_I                         (fused scale-add chain)

Layout per pair (A, B): hi tiles [128, 384] (A rows 0:128 in cols
0:192, B in 192:384), lo tiles [128, 192] (A rows 128:192 in
partitions 0:64, B in 64:128).  Matmuls run in bf16 (PSUM fp32).
Gram inner products: DVE/Pool elementwise multiply (bf16 2x) then a
one-hot selector matmul on the PE column-sums each product into a
per-molecule row of a PSUM accumulator; a per-level tensor_reduce
lands g directly in the [32, 9] solver layout.
"""

import sys

sys.path.insert(0, "/opt/trn_rl_repo")

import numpy as np

import concourse.bass as bass
import concourse.bacc as bacc
import concourse.tile as tile
from concourse import mybir
from concourse.bass_utils import run_bass_kernel_spmd

F32 = mybir.dt.float32
BF16 = mybir.dt.bfloat16
ALU = mybir.AluOpType
ACTF = mybir.ActivationFunctionType

NMOL, N, RANK = 512, 192, 4
NCORES = 8
MPC = NMOL // NCORES      # 64 molecules per core
NPAIR = MPC // 2          # 32 pairs
BLKP = 16                 # pairs per block (32 mols -> one batched solve)
NBLK = NPAIR // BLKP
HI, LO = 128, 64

# g[s] = <S_a, S_b> with a+b = s; level k (k=1..4) computes s = 2k-1, 2k.
G_PAIRS = {0: (0, 0), 1: (0, 1), 2: (1, 1), 3: (1, 2), 4: (2, 2),
           5: (2, 3), 6: (3, 3), 7: (3, 4), 8: (4, 4)}


def build_core_kernel():
    nc = bacc.Bacc(None, target_bir_lowering=False, enable_partition_id=False)
    D = nc.dram_tensor("D", [MPC, N, N], F32, kind="ExternalInput")
    P = nc.dram_tensor("P", [MPC, N, N], F32, kind="ExternalInput")
    R = nc.dram_tensor("Rm", [MPC, N, N], F32, kind="ExternalInput")
    OUT = nc.dram_tensor("OUT", [MPC, N, N], F32, kind="ExternalOutput")
    with tile.TileContext(nc) as tc:
        _body(nc, tc, D, P, R, OUT)
    nc.finalize()
    return nc


def _consts(nc, pool):
    c = {}
    # cb1: one-hot ones-column selector bank (col 31 = all-ones, fp32);
    # window cb1[:, 31-r : 63-r] routes a partials column-sum to gather
    # row r.
    cb1 = pool.tile([HI, 63], F32)
    nc.vector.memset(cb1, 0.0)
    nc.vector.memset(cb1[:, 31:32], 1.0)
    c["cb1"] = cb1
    # cb2: col 31 = upper-half ones, col 32 = lower-half ones; window at
    # row r sends partitions 0:64 to row r and 64:128 to row r+1.
    cb2 = pool.tile([HI, 64], F32)
    nc.vector.memset(cb2, 0.0)
    nc.vector.memset(cb2[0:LO, 31:32], 1.0)
    nc.vector.memset(cb2[LO:HI, 32:33], 1.0)
    c["cb2"] = cb2
    ones = pool.tile([HI, HI], F32)
    nc.vector.memset(ones, 1.0)
    c["ones"] = ones
    # selp: even partitions -> ones in cols 0:64, odd -> ones in 64:128.
    idp = pool.tile([HI, 1], mybir.dt.int32)
    nc.gpsimd.iota(idp, pattern=[[0, 1]], base=0, channel_multiplier=1)
    podd_i = pool.tile([HI, 1], mybir.dt.int32)
    nc.vector.tensor_scalar(out=podd_i, in0=idp, scalar1=1, scalar2=None,
                            op0=ALU.bitwise_and)
    podd = pool.tile([HI, 1], F32)
    nc.vector.tensor_scalar(out=podd, in0=podd_i, scalar1=1.0, scalar2=None,
                            op0=ALU.mult)
    pevn = pool.tile([HI, 1], F32)
    nc.vector.tensor_scalar(out=pevn, in0=podd, scalar1=-1.0, scalar2=1.0,
                            op0=ALU.mult, op1=ALU.add)
    selp = pool.tile([HI, HI], F32)
    nc.vector.tensor_scalar(out=selp[:, 0:LO], in0=ones[:, 0:LO],
                            scalar1=pevn, scalar2=None, op0=ALU.mult)
    nc.vector.tensor_scalar(out=selp[:, LO:HI], in0=ones[:, 0:LO],
                            scalar1=podd, scalar2=None, op0=ALU.mult)
    c["selp"] = selp
    # mask32[c, 4m+I] = (c == m); mask2[c, 4j+I] = (c in {2j, 2j+1})
    nm = 2 * BLKP
    mi = pool.tile([nm, 4 * nm], mybir.dt.int32)
    nc.gpsimd.iota(mi, pattern=[[-1, nm], [0, 4]], base=0,
                   channel_multiplier=1)
    mask32 = pool.tile([nm, 4 * nm], F32)
    nc.vector.tensor_scalar(out=mask32, in0=mi, scalar1=0, scalar2=None,
                            op0=ALU.is_equal)
    c["mask32"] = mask32
    mj = pool.tile([nm, 4 * BLKP], mybir.dt.int32)
    nc.gpsimd.iota(mj, pattern=[[-2, BLKP], [0, 4]], base=0,
                   channel_multiplier=1)
    m20 = pool.tile([nm, 4 * BLKP], F32)
    nc.vector.tensor_scalar(out=m20, in0=mj, scalar1=0, scalar2=None,
                            op0=ALU.is_equal)
    m21 = pool.tile([nm, 4 * BLKP], F32)
    nc.vector.tensor_scalar(out=m21, in0=mj, scalar1=1, scalar2=None,
                            op0=ALU.is_equal)
    mask2 = pool.tile([nm, 4 * BLKP], F32)
    nc.vector.tensor_add(mask2, m20, m21)
    c["mask2"] = mask2
    return c


def _body(nc, tc, D, P, R, OUT):
    import contextlib

    ctx = contextlib.ExitStack()
    with ctx:
        cpool = ctx.enter_context(tc.tile_pool(name="consts", bufs=1))
        sp = ctx.enter_context(tc.tile_pool(name="sp", bufs=BLKP + 2))
        tmp = ctx.enter_context(tc.tile_pool(name="tmp", bufs=3))
        tail = ctx.enter_context(tc.tile_pool(name="tail", bufs=2))
        ps_big = ctx.enter_context(tc.tile_pool(name="ps_big", bufs=3,
                                                space="PSUM"))
        ps_lo = ctx.enter_context(tc.tile_pool(name="ps_lo", bufs=3,
                                               space="PSUM"))
        ps_tl = ctx.enter_context(tc.tile_pool(name="ps_tl", bufs=1,
                                               space="PSUM"))
        C = _consts(nc, cpool)

        for b in range(NBLK):
            pairs = list(range(b * BLKP, (b + 1) * BLKP))
            st = {}
            for q in pairs:
                st[q] = _load_prep(nc, D, P, R, q, sp, tmp)
            _gram_level(nc, st, pairs, [0], tmp)
            for k in range(1, RANK + 1):
                _level(nc, st, pairs, k, sp, tmp, ps_big, ps_lo)
                _gram_level(nc, st, pairs, [2 * k - 1, 2 * k], tmp)
            g_sb = _gather(nc, st, pairs, C, tail, ps_tl)
            ybc = _tail(nc, pairs, C, tail, ps_tl, g_sb)
            for q in pairs:
                _combo(nc, st[q], q - b * BLKP, ybc, tmp)
            for q in pairs:
                _store(nc, OUT, q, st[q])


def _load_prep(nc, D, P, R, q, sp, tmp):
    """DMA loads, dDS = D - P (bf16), R cast for one pair."""
    mA, mB = 2 * q, 2 * q + 1
    stg = {}
    for nm, T in (("d", D), ("p", P), ("r", R)):
        sh = tmp.tile([HI, 2 * N], F32, name=f"{nm}sh", tag="stgh", bufs=6)
        sl = tmp.tile([HI, N], F32, name=f"{nm}sl", tag="stgl", bufs=6)
        nc.sync.dma_start(out=sh[:, 0:N], in_=T[mA, 0:HI, :])
        nc.sync.dma_start(out=sh[:, N:2 * N], in_=T[mB, 0:HI, :])
        nc.sync.dma_start(out=sl, in_=T[mA:mA + 2, HI:N, :])
        stg[nm] = (sh, sl)

    s0h = sp.tile([HI, 2 * N], BF16, tag="s0h")
    s0l = sp.tile([HI, N], BF16, tag="s0l")
    nc.vector.tensor_sub(s0h, stg["d"][0], stg["p"][0])
    nc.gpsimd.tensor_sub(s0l, stg["d"][1], stg["p"][1])
    rh = sp.tile([HI, 2 * N], BF16, tag="rh")
    rl = sp.tile([HI, N], BF16, tag="rl")
    nc.scalar.copy(rh, stg["r"][0])
    nc.gpsimd.tensor_copy(rl, stg["r"][1])
    partials = sp.tile([HI, 27], F32, tag="part")
    return {"sh": [s0h], "sl": [s0l], "rh": rh, "rl": rl, "part": partials}


def _mm_pair(nc, ps_big, ps_lo, lhs_hi, lhs_lo, rhs_hi, rhs_lo, tagp):
    """One 192x192 @ 192x192 product for both pair mols -> PSUM pair tiles.

    out[p,f] = sum_c lhs[c,p] rhs[c,f] per molecule; lhs must be symmetric
    (we pass S or R directly as lhsT).
    """
    ph = ps_big.tile([HI, 2 * N], F32, name=f"ph_{tagp}", tag="pbig")
    pl = ps_lo.tile([HI, N], F32, name=f"pl_{tagp}", tag="plo")
    for m, c0, p0 in ((0, 0, 0), (1, N, LO)):  # mol A, mol B
        hi_c = lhs_hi[:, c0:c0 + HI]          # lhs cols 0:128 (out rows hi)
        hi_cl = lhs_hi[:, c0 + HI:c0 + N]     # lhs cols 128:192 (out rows lo)
        lo_c = lhs_lo[p0:p0 + LO, 0:HI]
        lo_cl = lhs_lo[p0:p0 + LO, HI:N]
        rhi = rhs_hi[:, c0:c0 + N]
        rlo = rhs_lo[p0:p0 + LO, :]
        nc.tensor.matmul(ph[:, c0:c0 + N], lhsT=hi_c, rhs=rhi,
                         start=True, stop=False)
        nc.tensor.matmul(ph[:, c0:c0 + N], lhsT=lo_c, rhs=rlo,
                         start=False, stop=True)
        nc.tensor.matmul(pl[p0:p0 + LO, :], lhsT=hi_cl, rhs=rhi,
                         start=True, stop=False)
        nc.tensor.matmul(pl[p0:p0 + LO, :], lhsT=lo_cl, rhs=rlo,
                         start=False, stop=True)
    return ph, pl


def _drain(nc, eng, out, in_):
    if eng == "dve":
        nc.vector.tensor_copy(out, in_)
    else:
        nc.scalar.copy(out, in_)


# engine schedule for the per-level PSUM drains (dve/act only: gpsimd
# cannot touch PSUM)
T_BIG_ENG = ["act", "act", "act", "act"]
S_BIG_ENG = ["act", "act", "act", "act"]
T_SM_ENG = ["dve", "act", "dve", "act"]
S_SM_ENG = ["act", "dve", "act", "dve"]


def _level(nc, st, pairs, k, sp, tmp, ps_big, ps_lo):
    """Level k: T = S_{k-1} R then S_k = R T for every pair."""
    tps = {}
    for q in pairs:
        s = st[q]
        tps[q] = _mm_pair(nc, ps_big, ps_lo, s["sh"][k - 1], s["sl"][k - 1],
                          s["rh"], s["rl"], f"t{k}_{q}")
    tts = {}
    for q in pairs:
        th = tmp.tile([HI, 2 * N], BF16, name=f"th{k}_{q}", tag="th",
                      bufs=BLKP + 2)
        tl = tmp.tile([HI, N], BF16, name=f"tl{k}_{q}", tag="tl",
                      bufs=BLKP + 2)
        _drain(nc, T_BIG_ENG[k - 1], th, tps[q][0])
        _drain(nc, T_SM_ENG[k - 1], tl, tps[q][1])
        tts[q] = (th, tl)
    sps = {}
    for q in pairs:
        s = st[q]
        sps[q] = _mm_pair(nc, ps_big, ps_lo, s["rh"], s["rl"],
                          tts[q][0], tts[q][1], f"s{k}_{q}")
    for q in pairs:
        s = st[q]
        skh = sp.tile([HI, 2 * N], BF16, name=f"s{k}h", tag=f"s{k}h",
                      bufs=BLKP + 2)
        skl = sp.tile([HI, N], BF16, name=f"s{k}l", tag=f"s{k}l",
                      bufs=BLKP + 2)
        _drain(nc, S_BIG_ENG[k - 1], skh, sps[q][0])
        _drain(nc, S_SM_ENG[k - 1], skl, sps[q][1])
        s["sh"].append(skh)
        s["sl"].append(skl)


# evens (squares) hi tiles go to ACT; everything else is DVE TTR
def _gram_level(nc, st, pairs, svals, tmp):
    """g[s] = <S_a, S_b> partial sums into partials cols (hi-A: s,
    hi-B: 9+s, lo-pair: 18+s) via fused TTR on DVE / Square-accum on ACT."""
    for s in svals:
        a, bb = G_PAIRS[s]
        for q in pairs:
            stq = st[q]
            part = stq["part"]
            ah, bh = stq["sh"][a], stq["sh"][bb]
            al, bl = stq["sl"][a], stq["sl"][bb]
            if a == bb:
                for m, col in ((0, s), (1, 9 + s)):
                    junk = tmp.tile([HI, N], BF16, name="ja", tag="ja",
                                    bufs=3)
                    nc.scalar.activation(out=junk,
                                         in_=ah[:, m * N:(m + 1) * N],
                                         func=ACTF.Square,
                                         accum_out=part[:, col:col + 1])
            else:
                for m, col in ((0, s), (1, 9 + s)):
                    junk = tmp.tile([HI, N], BF16, name="jd", tag="jd",
                                    bufs=3)
                    nc.vector.scalar_tensor_tensor(
                        out=junk, in0=ah[:, m * N:(m + 1) * N], scalar=1.0,
                        in1=bh[:, m * N:(m + 1) * N],
                        op0=ALU.mult, op1=ALU.mult,
                        accum_out=part[:, col:col + 1])
            junk = tmp.tile([HI, N], BF16, name="jl", tag="jd", bufs=3)
            nc.vector.scalar_tensor_tensor(
                out=junk, in0=al, scalar=1.0, in1=bl,
                op0=ALU.mult, op1=ALU.mult, accum_out=part[:, 18 + s:19 + s])


def _gather(nc, st, pairs, C, tail, ps_tl):
    """Cross-partition reduce all pairs' partials into g_sb [32, 9]."""
    cb1, cb2 = C["cb1"], C["cb2"]
    gath = ps_tl.tile([2 * BLKP, 12], F32, tag="gath")
    nmm = 3 * len(pairs)
    i = 0
    for j, q in enumerate(pairs):
        part = st[q]["part"]
        rA = 2 * j
        for lhsT, rhs in (
            (cb1[:, 31 - rA:63 - rA], part[:, 0:9]),
            (cb1[:, 30 - rA:62 - rA], part[:, 9:18]),
            (cb2[:, 31 - rA:63 - rA], part[:, 18:27]),
        ):
            nc.tensor.matmul(gath[:, 0:9], lhsT=lhsT, rhs=rhs,
                             start=(i == 0), stop=(i == nmm - 1))
            i += 1
    g_sb = tail.tile([2 * BLKP, 9], F32, tag="g_sb")
    nc.vector.tensor_copy(g_sb, gath[:, 0:9])
    return g_sb


def _tail(nc, pairs, C, tail, ps_tl, g_sb):
    """Batched 4x4 solve from g, then broadcast -y to [128, *] columns."""
    nm = 2 * BLKP  # 32 molecules
    g = g_sb
    # Hankel assembly: h[s] = g[s] - 2 g[s+1] + g[s+2]; rhs c = diff(g)
    hs = tail.tile([nm, 7], F32, tag="hs")
    hm = tail.tile([nm, 7], F32, tag="hm")
    h = tail.tile([nm, 7], F32, tag="h")
    nc.vector.tensor_add(hs, g[:, 0:7], g[:, 2:9])
    nc.vector.tensor_scalar(out=hm, in0=g[:, 1:8], scalar1=-2.0, scalar2=None,
                            op0=ALU.mult)
    nc.vector.tensor_add(h, hs, hm)
    sv = tail.tile([nm, 14], F32, tag="sv")
    nc.vector.tensor_copy(sv[:, 0:4], h[:, 0:4])
    nc.vector.tensor_copy(sv[:, 4:7], h[:, 2:5])
    nc.vector.tensor_copy(sv[:, 7:9], h[:, 4:6])
    nc.vector.tensor_copy(sv[:, 9:10], h[:, 6:7])
    nc.vector.tensor_sub(sv[:, 10:14], g[:, 1:5], g[:, 0:4])

    ysb = _solve(nc, sv, tail, nm)
    ysn = tail.tile([nm, 4], F32, tag="ysn")
    nc.vector.tensor_scalar(out=ysn, in0=ysb, scalar1=-1.0, scalar2=None,
                            op0=ALU.mult)

    # broadcast -y to all partitions: cols 4m:(4m+4) per mol; cols
    # 128+4q:(128+4q+4) carry the packed-lo per-partition-half values.
    ones, selp = C["ones"], C["selp"]
    ysn_b = ysn.unsqueeze(1)
    yp = tail.tile([nm, 4 * nm], F32, tag="yp")
    nc.vector.tensor_mul(
        yp.rearrange("p (m i) -> p m i", i=4),
        C["mask32"].rearrange("p (m i) -> p m i", i=4),
        ysn_b.broadcast_to([nm, nm, 4]))
    yq = tail.tile([nm, 4 * BLKP], F32, tag="yq")
    nc.vector.tensor_mul(
        yq.rearrange("p (m i) -> p m i", i=4),
        C["mask2"].rearrange("p (m i) -> p m i", i=4),
        ysn_b.broadcast_to([nm, BLKP, 4]))
    ybp = ps_tl.tile([HI, N], F32, tag="ybp")
    nc.tensor.matmul(ybp[:, 0:4 * nm], lhsT=ones[0:nm, 0:HI], rhs=yp,
                     start=True, stop=True)
    nc.tensor.matmul(ybp[:, HI:HI + 4 * BLKP], lhsT=selp[0:nm, 0:HI],
                     rhs=yq, start=True, stop=True)
    ybc = tail.tile([HI, N], F32, tag="ybc")
    nc.scalar.copy(ybc, ybp)
    return ybc


def _solve(nc, sv, tail, nm):
    """Batched symmetric 4x4 Gauss elimination on [nm,1] column APs.

    sv cols: 0:a 1:b 2:c 3:d | 4:e 5:f 6:g | 7:h 8:i | 9:j | 10..13 r0..r3.
    Mirrors solve_batched_np (validated offline).
    """
    pp = tail.tile([nm, 4], F32, tag="pp")
    l3 = tail.tile([nm, 3], F32, tag="l3")
    tt = tail.tile([nm, 3], F32, tag="tt")
    ysb = tail.tile([nm, 4], F32, tag="ysb")

    ts = nc.vector.tensor_scalar
    sub = nc.vector.tensor_sub
    rec = nc.vector.reciprocal

    def upd(dst, src, scal, w=1):
        ts(out=tt[:, 0:w], in0=src, scalar1=scal, scalar2=None, op0=ALU.mult)
        sub(dst, dst, tt[:, 0:w])

    rec(pp[:, 0:1], sv[:, 0:1])
    ts(out=l3, in0=sv[:, 1:4], scalar1=pp[:, 0:1], scalar2=None, op0=ALU.mult)
    upd(sv[:, 4:7], l3, sv[:, 1:2], 3)          # (e,f,g) -= l*b
    upd(sv[:, 7:9], l3[:, 1:3], sv[:, 2:3], 2)  # (h,i) -= (l2,l3)*c
    upd(sv[:, 9:10], l3[:, 2:3], sv[:, 3:4])    # j -= l3*d
    upd(sv[:, 11:14], l3, sv[:, 10:11], 3)      # (r1,r2,r3) -= l*r0
    rec(pp[:, 1:2], sv[:, 4:5])
    ts(out=l3[:, 1:3], in0=sv[:, 5:7], scalar1=pp[:, 1:2], scalar2=None,
       op0=ALU.mult)
    upd(sv[:, 7:9], l3[:, 1:3], sv[:, 5:6], 2)
    upd(sv[:, 9:10], l3[:, 2:3], sv[:, 6:7])
    upd(sv[:, 12:14], l3[:, 1:3], sv[:, 11:12], 2)
    rec(pp[:, 2:3], sv[:, 7:8])
    ts(out=l3[:, 2:3], in0=sv[:, 8:9], scalar1=pp[:, 2:3], scalar2=None,
       op0=ALU.mult)
    upd(sv[:, 9:10], l3[:, 2:3], sv[:, 8:9])
    upd(sv[:, 13:14], l3[:, 2:3], sv[:, 12:13])
    rec(pp[:, 3:4], sv[:, 9:10])
    ts(out=ysb[:, 3:4], in0=sv[:, 13:14], scalar1=pp[:, 3:4], scalar2=None,
       op0=ALU.mult)
    upd(sv[:, 12:13], sv[:, 8:9], ysb[:, 3:4])
    ts(out=ysb[:, 2:3], in0=sv[:, 12:13], scalar1=pp[:, 2:3], scalar2=None,
       op0=ALU.mult)
    upd(sv[:, 11:12], sv[:, 5:6], ysb[:, 2:3])
    upd(sv[:, 11:12], sv[:, 6:7], ysb[:, 3:4])
    ts(out=ysb[:, 1:2], in0=sv[:, 11:12], scalar1=pp[:, 1:2], scalar2=None,
       op0=ALU.mult)
    upd(sv[:, 10:11], sv[:, 1:2], ysb[:, 1:2])
    upd(sv[:, 10:11], sv[:, 2:3], ysb[:, 2:3])
    upd(sv[:, 10:11], sv[:, 3:4], ysb[:, 3:4])
    ts(out=ysb[:, 0:1], in0=sv[:, 10:11], scalar1=pp[:, 0:1], scalar2=None,
       op0=ALU.mult)
    return ysb


def _combo(nc, stq, j, ybc, tmp):
    """acc = sum_I (-y_I) S_I via a fused scale-accumulate (STT) chain."""
    mA, mB = 2 * j, 2 * j + 1
    ah = tmp.tile([HI, 2 * N], F32, name="acch", tag="acch", bufs=4)
    al = tmp.tile([HI, N], F32, name="accl", tag="accl", bufs=4)
    for m, c0 in ((mA, 0), (mB, N)):
        u0 = tmp.tile([HI, N], BF16, name="cu0", tag="cu", bufs=6)
        nc.vector.tensor_scalar(out=u0, in0=stq["sh"][0][:, c0:c0 + N],
                                scalar1=ybc[:, 4 * m:4 * m + 1],
                                scalar2=None, op0=ALU.mult)
        for I in (1, 2):
            u1 = tmp.tile([HI, N], BF16, name="cu1", tag="cu", bufs=6)
            nc.vector.scalar_tensor_tensor(
                out=u1, in0=stq["sh"][I][:, c0:c0 + N],
                scalar=ybc[:, 4 * m + I:4 * m + I + 1], in1=u0,
                op0=ALU.mult, op1=ALU.add)
            u0 = u1
        nc.vector.scalar_tensor_tensor(
            out=ah[:, c0:c0 + N], in0=stq["sh"][3][:, c0:c0 + N],
            scalar=ybc[:, 4 * m + 3:4 * m + 4], in1=u0,
            op0=ALU.mult, op1=ALU.add)
    u0 = tmp.tile([HI, N], BF16, name="cul0", tag="cu", bufs=6)
    nc.vector.tensor_scalar(out=u0, in0=stq["sl"][0],
                            scalar1=ybc[:, HI + 4 * j:HI + 4 * j + 1],
                            scalar2=None, op0=ALU.mult)
    for I in (1, 2):
        u1 = tmp.tile([HI, N], BF16, name="cul1", tag="cu", bufs=6)
        nc.vector.scalar_tensor_tensor(
            out=u1, in0=stq["sl"][I],
            scalar=ybc[:, HI + 4 * j + I:HI + 4 * j + I + 1], in1=u0,
            op0=ALU.mult, op1=ALU.add)
        u0 = u1
    nc.vector.scalar_tensor_tensor(
        out=al, in0=stq["sl"][3],
        scalar=ybc[:, HI + 4 * j + 3:HI + 4 * j + 4], in1=u0,
        op0=ALU.mult, op1=ALU.add)
    stq["acc"] = (ah, al)


def _store(nc, OUT, q, stq):
    mA, mB = 2 * q, 2 * q + 1
    ah, al = stq["acc"]
    nc.sync.dma_start(out=OUT[mA, 0:HI, :], in_=ah[:, 0:N])
    nc.sync.dma_start(out=OUT[mB, 0:HI, :], in_=ah[:, N:2 * N])
    nc.sync.dma_start(out=OUT[mA:mA + 2, HI:N, :], in_=al)


_NC_CACHE = None


def _get_nc():
    global _NC_CACHE
    if _NC_CACHE is None:
        _NC_CACHE = build_core_kernel()
    return _NC_CACHE


def kernel(D, P, R, max_rank=4, _trace=False):
    D = np.ascontiguousarray(D, dtype=np.float32)
    P = np.ascontiguousarray(P, dtype=np.float32)
    R = np.ascontiguousarray(R, dtype=np.float32)
    nc = _get_nc()
    in_maps = []
    for i in range(NCORES):
        sl = slice(i * MPC, (i + 1) * MPC)
        in_maps.append({"D": D[sl], "P": P[sl], "Rm": R[sl]})
    res = run_bass_kernel_spmd(nc, in_maps, core_ids=list(range(NCORES)),
                               trace=_trace)
    out = np.concatenate([r["OUT"] for r in res.results], axis=0)
    if _trace:
        kernel.last_exec_time_ns = res.exec_time_ns
        kernel.last_trace = res.instructions_and_trace
    return out


if __name__ == "__main__":
    import tempfile
    from concourse.bass_utils import compile_bass_kernel
    nc = build_core_kernel()
    print("build OK")
    if "--compile" in sys.argv:
        td = tempfile.mkdtemp()
        print("NEFF:", compile_bass_kernel(nc, td))


# revision 21
# speedup vs baseline: 4.9954x; 1.1539x over previous
"""XL-BOMD rank-4 Krylov propagation (EnergyXL) on 8 TRN2 NeuronCores.

Data-parallel over molecules: 512 mols -> 64 per core, processed in
pairs.  Per molecule (N=192, rank=4) the reference computes

    out = -V (W^T W)^{-1} W^T dDS,   W = F(V) = R V R - V

over the Gram-Schmidt basis V of the Krylov space K_4(dDS).  The
output is invariant under ANY invertible change of basis of K_4
(W is linear in V), so we use the raw power iterates S_k = R^k dDS R^k
directly:

    S_0 = D - P;  S_k = R S_{k-1} R              (8 bf16 PE products/mol)
    g[s] = <S_a, S_b>  (a+b = s, s = 0..8)       (Gram is Hankel: 9 ips)
    O[I,J] = g[I+J+2] - 2 g[I+J+1] + g[I+J],  c[J] = g[J+1] - g[J]
    y = O^{-1} c   (batched 4x4 Gauss over 32-mol blocks)
    out = -sum_I y_I S_I                         (fused scale-add chain)

Layout per pair (A, B): hi tiles [128, 384] (A rows 0:128 in cols
0:192, B in 192:384), lo tiles [128, 192] (A rows 128:192 in
partitions 0:64, B in 64:128).  Matmuls run in bf16 (PSUM fp32).
Gram inner products: DVE/Pool elementwise multiply (bf16 2x) then a
one-hot selector matmul on the PE column-sums each product into a
per-molecule row of a PSUM accumulator; a per-level tensor_reduce
lands g directly in the [32, 9] solver layout.
"""

import sys

sys.path.insert(0, "/opt/trn_rl_repo")

import numpy as np

import concourse.bass as bass
import concourse.bacc as bacc
import concourse.tile as tile
from concourse import mybir
from concourse.bass_utils import run_bass_kernel_spmd

F32 = mybir.dt.float32
BF16 = mybir.dt.bfloat16
ALU = mybir.AluOpType
ACTF = mybir.ActivationFunctionType

NMOL, N, RANK = 512, 192, 4
NCORES = 8
MPC = NMOL // NCORES      # 64 molecules per core
NPAIR = MPC // 2          # 32 pairs
BLKP = 16                 # pairs per block (32 mols -> one batched solve)
NBLK = NPAIR // BLKP
HI, LO = 128, 64

# g[s] = <S_a, S_b> with a+b = s; level k (k=1..4) computes s = 2k-1, 2k.
G_PAIRS = {0: (0, 0), 1: (0, 1), 2: (1, 1), 3: (1, 2), 4: (2, 2),
           5: (2, 3), 6: (3, 3), 7: (3, 4), 8: (4, 4)}


def build_core_kernel():
    nc = bacc.Bacc(None, target_bir_lowering=False, enable_partition_id=False)
    D = nc.dram_tensor("D", [MPC, N, N], F32, kind="ExternalInput")
    P = nc.dram_tensor("P", [MPC, N, N], F32, kind="ExternalInput")
    R = nc.dram_tensor("Rm", [MPC, N, N], F32, kind="ExternalInput")
    OUT = nc.dram_tensor("OUT", [MPC, N, N], F32, kind="ExternalOutput")
    with tile.TileContext(nc) as tc:
        _body(nc, tc, D, P, R, OUT)
    nc.finalize()
    return nc


def _consts(nc, pool):
    c = {}
    # cb1: one-hot ones-column selector bank (col 31 = all-ones, fp32);
    # window cb1[:, 31-r : 63-r] routes a partials column-sum to gather
    # row r.
    cb1 = pool.tile([HI, 63], F32)
    nc.vector.memset(cb1, 0.0)
    nc.vector.memset(cb1[:, 31:32], 1.0)
    c["cb1"] = cb1
    # cb2: col 31 = upper-half ones, col 32 = lower-half ones; window at
    # row r sends partitions 0:64 to row r and 64:128 to row r+1.
    cb2 = pool.tile([HI, 64], F32)
    nc.vector.memset(cb2, 0.0)
    nc.vector.memset(cb2[0:LO, 31:32], 1.0)
    nc.vector.memset(cb2[LO:HI, 32:33], 1.0)
    c["cb2"] = cb2
    ones = pool.tile([HI, HI], F32)
    nc.vector.memset(ones, 1.0)
    c["ones"] = ones
    # selp: even partitions -> ones in cols 0:64, odd -> ones in 64:128.
    idp = pool.tile([HI, 1], mybir.dt.int32)
    nc.gpsimd.iota(idp, pattern=[[0, 1]], base=0, channel_multiplier=1)
    podd_i = pool.tile([HI, 1], mybir.dt.int32)
    nc.vector.tensor_scalar(out=podd_i, in0=idp, scalar1=1, scalar2=None,
                            op0=ALU.bitwise_and)
    podd = pool.tile([HI, 1], F32)
    nc.vector.tensor_scalar(out=podd, in0=podd_i, scalar1=1.0, scalar2=None,
                            op0=ALU.mult)
    pevn = pool.tile([HI, 1], F32)
    nc.vector.tensor_scalar(out=pevn, in0=podd, scalar1=-1.0, scalar2=1.0,
                            op0=ALU.mult, op1=ALU.add)
    selp = pool.tile([HI, HI], F32)
    nc.vector.tensor_scalar(out=selp[:, 0:LO], in0=ones[:, 0:LO],
                            scalar1=pevn, scalar2=None, op0=ALU.mult)
    nc.vector.tensor_scalar(out=selp[:, LO:HI], in0=ones[:, 0:LO],
                            scalar1=podd, scalar2=None, op0=ALU.mult)
    c["selp"] = selp
    # mask32[c, 4m+I] = (c == m); mask2[c, 4j+I] = (c in {2j, 2j+1})
    nm = 2 * BLKP
    mi = pool.tile([nm, 4 * nm], mybir.dt.int32)
    nc.gpsimd.iota(mi, pattern=[[-1, nm], [0, 4]], base=0,
                   channel_multiplier=1)
    mask32 = pool.tile([nm, 4 * nm], F32)
    nc.vector.tensor_scalar(out=mask32, in0=mi, scalar1=0, scalar2=None,
                            op0=ALU.is_equal)
    c["mask32"] = mask32
    mj = pool.tile([nm, 4 * BLKP], mybir.dt.int32)
    nc.gpsimd.iota(mj, pattern=[[-2, BLKP], [0, 4]], base=0,
                   channel_multiplier=1)
    m20 = pool.tile([nm, 4 * BLKP], F32)
    nc.vector.tensor_scalar(out=m20, in0=mj, scalar1=0, scalar2=None,
                            op0=ALU.is_equal)
    m21 = pool.tile([nm, 4 * BLKP], F32)
    nc.vector.tensor_scalar(out=m21, in0=mj, scalar1=1, scalar2=None,
                            op0=ALU.is_equal)
    mask2 = pool.tile([nm, 4 * BLKP], F32)
    nc.vector.tensor_add(mask2, m20, m21)
    c["mask2"] = mask2
    return c


def _body(nc, tc, D, P, R, OUT):
    import contextlib

    ctx = contextlib.ExitStack()
    with ctx:
        cpool = ctx.enter_context(tc.tile_pool(name="consts", bufs=1))
        sp = ctx.enter_context(tc.tile_pool(name="sp", bufs=BLKP + 2))
        tmp = ctx.enter_context(tc.tile_pool(name="tmp", bufs=3))
        tail = ctx.enter_context(tc.tile_pool(name="tail", bufs=2))
        ps_big = ctx.enter_context(tc.tile_pool(name="ps_big", bufs=3,
                                                space="PSUM"))
        ps_lo = ctx.enter_context(tc.tile_pool(name="ps_lo", bufs=3,
                                               space="PSUM"))
        ps_tl = ctx.enter_context(tc.tile_pool(name="ps_tl", bufs=1,
                                               space="PSUM"))
        C = _consts(nc, cpool)

        for b in range(NBLK):
            pairs = list(range(b * BLKP, (b + 1) * BLKP))
            st = {}
            for q in pairs:
                st[q] = _load_prep(nc, D, P, R, q, sp, tmp)
            # gram for level k-1 is emitted after level k's products so the
            # ACT squares queue behind the drains they'd otherwise delay
            for k in range(1, RANK + 1):
                _level(nc, st, pairs, k, sp, tmp, ps_big, ps_lo)
                _gram_level(nc, st, pairs,
                            [0] if k == 1 else [2 * k - 3, 2 * k - 2], tmp)
            _gram_level(nc, st, pairs, [2 * RANK - 1, 2 * RANK], tmp)
            g_sb = _gather(nc, st, pairs, C, tail, ps_tl)
            ybc = _tail(nc, pairs, C, tail, ps_tl, g_sb)
            for q in pairs:
                _combo(nc, st[q], q - b * BLKP, ybc, tmp)
            for q in pairs:
                _store(nc, OUT, q, st[q])


def _load_prep(nc, D, P, R, q, sp, tmp):
    """DMA loads, dDS = D - P (bf16), R cast for one pair."""
    mA, mB = 2 * q, 2 * q + 1
    stg = {}
    for nm, T in (("d", D), ("p", P), ("r", R)):
        sh = tmp.tile([HI, 2 * N], F32, name=f"{nm}sh", tag="stgh", bufs=10)
        sl = tmp.tile([HI, N], F32, name=f"{nm}sl", tag="stgl", bufs=10)
        nc.sync.dma_start(out=sh.rearrange("p (m c) -> p m c", m=2),
                          in_=T[mA:mA + 2, 0:HI, :].transpose([1, 0, 2]))
        nc.sync.dma_start(out=sl, in_=T[mA:mA + 2, HI:N, :])
        stg[nm] = (sh, sl)

    s0h = sp.tile([HI, 2 * N], BF16, tag="s0h")
    s0l = sp.tile([HI, N], BF16, tag="s0l")
    nc.vector.tensor_sub(s0h, stg["d"][0], stg["p"][0])
    nc.gpsimd.tensor_sub(s0l, stg["d"][1], stg["p"][1])
    rh = sp.tile([HI, 2 * N], BF16, tag="rh")
    rl = sp.tile([HI, N], BF16, tag="rl")
    nc.scalar.copy(rh, stg["r"][0])
    nc.gpsimd.tensor_copy(rl, stg["r"][1])
    rbd = sp.tile([HI, HI], BF16, tag="rbd")
    nc.gpsimd.memset(rbd, 0.0)
    nc.gpsimd.tensor_copy(rbd[0:LO, 0:LO], rl[0:LO, HI:N])
    nc.gpsimd.tensor_copy(rbd[LO:HI, LO:HI], rl[LO:HI, HI:N])
    partials = sp.tile([HI, 27], F32, tag="part")
    return {"sh": [s0h], "sl": [s0l], "rh": rh, "rl": rl, "rbd": rbd,
            "part": partials}


def _mm_pair(nc, ps_big, ps_lo, lhs_hi, lhs_lo, bd, rhs_hi, rhs_lo, tagp):
    """One 192x192 @ 192x192 product for both pair mols -> PSUM pair tiles.

    out[p,f] = sum_c lhs[c,p] rhs[c,f] per molecule; lhs must be symmetric
    (we pass S or R directly as lhsT).  bd is the block-diagonal packing of
    the two mols' (c-lo, p-lo) corner chunks so that corner runs as ONE
    matmul over the packed-lo partitions.
    """
    ph = ps_big.tile([HI, 2 * N], F32, name=f"ph_{tagp}", tag="pbig")
    pl = ps_lo.tile([HI, N], F32, name=f"pl_{tagp}", tag="plo")
    for m, c0, p0 in ((0, 0, 0), (1, N, LO)):  # mol A, mol B
        hi_c = lhs_hi[:, c0:c0 + HI]          # lhs cols 0:128 (out rows hi)
        hi_cl = lhs_hi[:, c0 + HI:c0 + N]     # lhs cols 128:192 (out rows lo)
        lo_c = lhs_lo[p0:p0 + LO, 0:HI]
        rhi = rhs_hi[:, c0:c0 + N]
        rlo = rhs_lo[p0:p0 + LO, :]
        nc.tensor.matmul(ph[:, c0:c0 + N], lhsT=hi_c, rhs=rhi,
                         start=True, stop=False)
        nc.tensor.matmul(ph[:, c0:c0 + N], lhsT=lo_c, rhs=rlo,
                         start=False, stop=True)
        nc.tensor.matmul(pl[p0:p0 + LO, :], lhsT=hi_cl, rhs=rhi,
                         start=True, stop=False)
    nc.tensor.matmul(pl[:, :], lhsT=bd, rhs=rhs_lo,
                     start=False, stop=True, skip_group_check=True)
    return ph, pl


def _drain(nc, eng, out, in_):
    if eng == "dve":
        nc.vector.tensor_copy(out, in_)
    else:
        nc.scalar.copy(out, in_)


# engine schedule for the per-level PSUM drains (dve/act only: gpsimd
# cannot touch PSUM)
T_BIG_ENG = ["act", "act", "act", "act"]
S_BIG_ENG = ["act", "act", "act", "act"]
T_SM_ENG = ["dve", "dve", "dve", "dve"]
S_SM_ENG = ["dve", "dve", "dve", "dve"]


def _level(nc, st, pairs, k, sp, tmp, ps_big, ps_lo):
    """Level k: T = S_{k-1} R then S_k = R T for every pair."""
    bds = {}
    for q in pairs:
        s = st[q]
        bd = tmp.tile([HI, HI], BF16, name=f"bdt{k}_{q}", tag="bdt",
                      bufs=BLKP + 2)
        sl = s["sl"][k - 1]
        nc.gpsimd.memset(bd, 0.0)
        nc.gpsimd.tensor_copy(bd[0:LO, 0:LO], sl[0:LO, HI:N])
        nc.gpsimd.tensor_copy(bd[LO:HI, LO:HI], sl[LO:HI, HI:N])
        bds[q] = bd
    tps = {}
    for q in pairs:
        s = st[q]
        tps[q] = _mm_pair(nc, ps_big, ps_lo, s["sh"][k - 1], s["sl"][k - 1],
                          bds[q], s["rh"], s["rl"], f"t{k}_{q}")
    tts = {}
    for q in pairs:
        th = tmp.tile([HI, 2 * N], BF16, name=f"th{k}_{q}", tag="th",
                      bufs=BLKP + 2)
        tl = tmp.tile([HI, N], BF16, name=f"tl{k}_{q}", tag="tl",
                      bufs=BLKP + 2)
        _drain(nc, T_BIG_ENG[k - 1], th, tps[q][0])
        _drain(nc, T_SM_ENG[k - 1], tl, tps[q][1])
        tts[q] = (th, tl)
    sps = {}
    for q in pairs:
        s = st[q]
        sps[q] = _mm_pair(nc, ps_big, ps_lo, s["rh"], s["rl"], s["rbd"],
                          tts[q][0], tts[q][1], f"s{k}_{q}")
    for q in pairs:
        s = st[q]
        skh = sp.tile([HI, 2 * N], BF16, name=f"s{k}h", tag=f"s{k}h",
                      bufs=BLKP + 2)
        skl = sp.tile([HI, N], BF16, name=f"s{k}l", tag=f"s{k}l",
                      bufs=BLKP + 2)
        _drain(nc, S_BIG_ENG[k - 1], skh, sps[q][0])
        _drain(nc, S_SM_ENG[k - 1], skl, sps[q][1])
        s["sh"].append(skh)
        s["sl"].append(skl)


# evens (squares) hi tiles go to ACT; everything else is DVE TTR
def _gram_level(nc, st, pairs, svals, tmp):
    """g[s] = <S_a, S_b> partial sums into partials cols (hi-A: s,
    hi-B: 9+s, lo-pair: 18+s) via fused TTR on DVE / Square-accum on ACT."""
    for s in svals:
        a, bb = G_PAIRS[s]
        for q in pairs:
            stq = st[q]
            part = stq["part"]
            ah, bh = stq["sh"][a], stq["sh"][bb]
            al, bl = stq["sl"][a], stq["sl"][bb]
            if a == bb:
                for m, col in ((0, s), (1, 9 + s)):
                    junk = tmp.tile([HI, N], BF16, name="ja", tag="ja",
                                    bufs=3)
                    nc.scalar.activation(out=junk,
                                         in_=ah[:, m * N:(m + 1) * N],
                                         func=ACTF.Square,
                                         accum_out=part[:, col:col + 1])
            else:
                for m, col in ((0, s), (1, 9 + s)):
                    junk = tmp.tile([HI, N], BF16, name="jd", tag="jd",
                                    bufs=3)
                    nc.vector.scalar_tensor_tensor(
                        out=junk, in0=ah[:, m * N:(m + 1) * N], scalar=1.0,
                        in1=bh[:, m * N:(m + 1) * N],
                        op0=ALU.mult, op1=ALU.mult,
                        accum_out=part[:, col:col + 1])
            junk = tmp.tile([HI, N], BF16, name="jl", tag="jd", bufs=3)
            nc.vector.scalar_tensor_tensor(
                out=junk, in0=al, scalar=1.0, in1=bl,
                op0=ALU.mult, op1=ALU.mult, accum_out=part[:, 18 + s:19 + s])


def _gather(nc, st, pairs, C, tail, ps_tl):
    """Cross-partition reduce all pairs' partials into g_sb [32, 9]."""
    cb1, cb2 = C["cb1"], C["cb2"]
    gath = ps_tl.tile([2 * BLKP, 12], F32, tag="gath")
    nmm = 3 * len(pairs)
    i = 0
    for j, q in enumerate(pairs):
        part = st[q]["part"]
        rA = 2 * j
        for lhsT, rhs in (
            (cb1[:, 31 - rA:63 - rA], part[:, 0:9]),
            (cb1[:, 30 - rA:62 - rA], part[:, 9:18]),
            (cb2[:, 31 - rA:63 - rA], part[:, 18:27]),
        ):
            nc.tensor.matmul(gath[:, 0:9], lhsT=lhsT, rhs=rhs,
                             start=(i == 0), stop=(i == nmm - 1))
            i += 1
    g_sb = tail.tile([2 * BLKP, 9], F32, tag="g_sb")
    nc.vector.tensor_copy(g_sb, gath[:, 0:9])
    return g_sb


def _tail(nc, pairs, C, tail, ps_tl, g_sb):
    """Batched 4x4 solve from g, then broadcast -y to [128, *] columns."""
    nm = 2 * BLKP  # 32 molecules
    g = g_sb
    # Hankel assembly: h[s] = g[s] - 2 g[s+1] + g[s+2]; rhs c = diff(g)
    hs = tail.tile([nm, 7], F32, tag="hs")
    hm = tail.tile([nm, 7], F32, tag="hm")
    h = tail.tile([nm, 7], F32, tag="h")
    nc.vector.tensor_add(hs, g[:, 0:7], g[:, 2:9])
    nc.vector.tensor_scalar(out=hm, in0=g[:, 1:8], scalar1=-2.0, scalar2=None,
                            op0=ALU.mult)
    nc.vector.tensor_add(h, hs, hm)
    sv = tail.tile([nm, 14], F32, tag="sv")
    nc.vector.tensor_copy(sv[:, 0:4], h[:, 0:4])
    nc.vector.tensor_copy(sv[:, 4:7], h[:, 2:5])
    nc.vector.tensor_copy(sv[:, 7:9], h[:, 4:6])
    nc.vector.tensor_copy(sv[:, 9:10], h[:, 6:7])
    nc.vector.tensor_sub(sv[:, 10:14], g[:, 1:5], g[:, 0:4])

    ysb = _solve(nc, sv, tail, nm)
    ysn = tail.tile([nm, 4], F32, tag="ysn")
    nc.vector.tensor_scalar(out=ysn, in0=ysb, scalar1=-1.0, scalar2=None,
                            op0=ALU.mult)

    # broadcast -y to all partitions: cols 4m:(4m+4) per mol; cols
    # 128+4q:(128+4q+4) carry the packed-lo per-partition-half values.
    ones, selp = C["ones"], C["selp"]
    ysn_b = ysn.unsqueeze(1)
    yp = tail.tile([nm, 4 * nm], F32, tag="yp")
    nc.vector.tensor_mul(
        yp.rearrange("p (m i) -> p m i", i=4),
        C["mask32"].rearrange("p (m i) -> p m i", i=4),
        ysn_b.broadcast_to([nm, nm, 4]))
    yq = tail.tile([nm, 4 * BLKP], F32, tag="yq")
    nc.vector.tensor_mul(
        yq.rearrange("p (m i) -> p m i", i=4),
        C["mask2"].rearrange("p (m i) -> p m i", i=4),
        ysn_b.broadcast_to([nm, BLKP, 4]))
    ybp = ps_tl.tile([HI, N], F32, tag="ybp")
    nc.tensor.matmul(ybp[:, 0:4 * nm], lhsT=ones[0:nm, 0:HI], rhs=yp,
                     start=True, stop=True)
    nc.tensor.matmul(ybp[:, HI:HI + 4 * BLKP], lhsT=selp[0:nm, 0:HI],
                     rhs=yq, start=True, stop=True)
    ybc = tail.tile([HI, N], F32, tag="ybc")
    nc.scalar.copy(ybc, ybp)
    return ybc


def _solve(nc, sv, tail, nm):
    """Batched symmetric 4x4 Gauss elimination on [nm,1] column APs.

    sv cols: 0:a 1:b 2:c 3:d | 4:e 5:f 6:g | 7:h 8:i | 9:j | 10..13 r0..r3.
    Mirrors solve_batched_np (validated offline).
    """
    pp = tail.tile([nm, 4], F32, tag="pp")
    l3 = tail.tile([nm, 3], F32, tag="l3")
    tt = tail.tile([nm, 3], F32, tag="tt")
    ysb = tail.tile([nm, 4], F32, tag="ysb")

    ts = nc.vector.tensor_scalar
    sub = nc.vector.tensor_sub
    rec = nc.vector.reciprocal

    def upd(dst, src, scal, w=1):
        ts(out=tt[:, 0:w], in0=src, scalar1=scal, scalar2=None, op0=ALU.mult)
        sub(dst, dst, tt[:, 0:w])

    rec(pp[:, 0:1], sv[:, 0:1])
    ts(out=l3, in0=sv[:, 1:4], scalar1=pp[:, 0:1], scalar2=None, op0=ALU.mult)
    upd(sv[:, 4:7], l3, sv[:, 1:2], 3)          # (e,f,g) -= l*b
    upd(sv[:, 7:9], l3[:, 1:3], sv[:, 2:3], 2)  # (h,i) -= (l2,l3)*c
    upd(sv[:, 9:10], l3[:, 2:3], sv[:, 3:4])    # j -= l3*d
    upd(sv[:, 11:14], l3, sv[:, 10:11], 3)      # (r1,r2,r3) -= l*r0
    rec(pp[:, 1:2], sv[:, 4:5])
    ts(out=l3[:, 1:3], in0=sv[:, 5:7], scalar1=pp[:, 1:2], scalar2=None,
       op0=ALU.mult)
    upd(sv[:, 7:9], l3[:, 1:3], sv[:, 5:6], 2)
    upd(sv[:, 9:10], l3[:, 2:3], sv[:, 6:7])
    upd(sv[:, 12:14], l3[:, 1:3], sv[:, 11:12], 2)
    rec(pp[:, 2:3], sv[:, 7:8])
    ts(out=l3[:, 2:3], in0=sv[:, 8:9], scalar1=pp[:, 2:3], scalar2=None,
       op0=ALU.mult)
    upd(sv[:, 9:10], l3[:, 2:3], sv[:, 8:9])
    upd(sv[:, 13:14], l3[:, 2:3], sv[:, 12:13])
    rec(pp[:, 3:4], sv[:, 9:10])
    ts(out=ysb[:, 3:4], in0=sv[:, 13:14], scalar1=pp[:, 3:4], scalar2=None,
       op0=ALU.mult)
    upd(sv[:, 12:13], sv[:, 8:9], ysb[:, 3:4])
    ts(out=ysb[:, 2:3], in0=sv[:, 12:13], scalar1=pp[:, 2:3], scalar2=None,
       op0=ALU.mult)
    upd(sv[:, 11:12], sv[:, 5:6], ysb[:, 2:3])
    upd(sv[:, 11:12], sv[:, 6:7], ysb[:, 3:4])
    ts(out=ysb[:, 1:2], in0=sv[:, 11:12], scalar1=pp[:, 1:2], scalar2=None,
       op0=ALU.mult)
    upd(sv[:, 10:11], sv[:, 1:2], ysb[:, 1:2])
    upd(sv[:, 10:11], sv[:, 2:3], ysb[:, 2:3])
    upd(sv[:, 10:11], sv[:, 3:4], ysb[:, 3:4])
    ts(out=ysb[:, 0:1], in0=sv[:, 10:11], scalar1=pp[:, 0:1], scalar2=None,
       op0=ALU.mult)
    return ysb


def _combo(nc, stq, j, ybc, tmp):
    """acc = sum_I (-y_I) S_I via a fused scale-accumulate (STT) chain."""
    mA, mB = 2 * j, 2 * j + 1
    ah = tmp.tile([HI, 2 * N], F32, name="acch", tag="acch", bufs=4)
    al = tmp.tile([HI, N], F32, name="accl", tag="accl", bufs=4)
    for m, c0 in ((mA, 0), (mB, N)):
        u0 = tmp.tile([HI, N], BF16, name="cu0", tag="cu", bufs=6)
        nc.vector.tensor_scalar(out=u0, in0=stq["sh"][0][:, c0:c0 + N],
                                scalar1=ybc[:, 4 * m:4 * m + 1],
                                scalar2=None, op0=ALU.mult)
        for I in (1, 2):
            u1 = tmp.tile([HI, N], BF16, name="cu1", tag="cu", bufs=6)
            nc.vector.scalar_tensor_tensor(
                out=u1, in0=stq["sh"][I][:, c0:c0 + N],
                scalar=ybc[:, 4 * m + I:4 * m + I + 1], in1=u0,
                op0=ALU.mult, op1=ALU.add)
            u0 = u1
        nc.vector.scalar_tensor_tensor(
            out=ah[:, c0:c0 + N], in0=stq["sh"][3][:, c0:c0 + N],
            scalar=ybc[:, 4 * m + 3:4 * m + 4], in1=u0,
            op0=ALU.mult, op1=ALU.add)
    u0 = tmp.tile([HI, N], BF16, name="cul0", tag="cu", bufs=6)
    nc.vector.tensor_scalar(out=u0, in0=stq["sl"][0],
                            scalar1=ybc[:, HI + 4 * j:HI + 4 * j + 1],
                            scalar2=None, op0=ALU.mult)
    for I in (1, 2):
        u1 = tmp.tile([HI, N], BF16, name="cul1", tag="cu", bufs=6)
        nc.vector.scalar_tensor_tensor(
            out=u1, in0=stq["sl"][I],
            scalar=ybc[:, HI + 4 * j + I:HI + 4 * j + I + 1], in1=u0,
            op0=ALU.mult, op1=ALU.add)
        u0 = u1
    nc.vector.scalar_tensor_tensor(
        out=al, in0=stq["sl"][3],
        scalar=ybc[:, HI + 4 * j + 3:HI + 4 * j + 4], in1=u0,
        op0=ALU.mult, op1=ALU.add)
    stq["acc"] = (ah, al)


def _store(nc, OUT, q, stq):
    mA = 2 * q
    ah, al = stq["acc"]
    nc.sync.dma_start(out=OUT[mA:mA + 2, 0:HI, :].transpose([1, 0, 2]),
                      in_=ah.rearrange("p (m c) -> p m c", m=2))
    nc.sync.dma_start(out=OUT[mA:mA + 2, HI:N, :], in_=al)


_NC_CACHE = None


def _get_nc():
    global _NC_CACHE
    if _NC_CACHE is None:
        _NC_CACHE = build_core_kernel()
    return _NC_CACHE


def kernel(D, P, R, max_rank=4, _trace=False):
    D = np.ascontiguousarray(D, dtype=np.float32)
    P = np.ascontiguousarray(P, dtype=np.float32)
    R = np.ascontiguousarray(R, dtype=np.float32)
    nc = _get_nc()
    in_maps = []
    for i in range(NCORES):
        sl = slice(i * MPC, (i + 1) * MPC)
        in_maps.append({"D": D[sl], "P": P[sl], "Rm": R[sl]})
    res = run_bass_kernel_spmd(nc, in_maps, core_ids=list(range(NCORES)),
                               trace=_trace)
    out = np.concatenate([r["OUT"] for r in res.results], axis=0)
    if _trace:
        kernel.last_exec_time_ns = res.exec_time_ns
        kernel.last_trace = res.instructions_and_trace
    return out


if __name__ == "__main__":
    import tempfile
    from concourse.bass_utils import compile_bass_kernel
    nc = build_core_kernel()
    print("build OK")
    if "--compile" in sys.argv:
        td = tempfile.mkdtemp()
        print("NEFF:", compile_bass_kernel(nc, td))


# revision 22
# speedup vs baseline: 5.3509x; 1.0712x over previous
"""XL-BOMD rank-4 Krylov propagation (EnergyXL) on 8 TRN2 NeuronCores.

Data-parallel over molecules: 512 mols -> 64 per core, processed in
pairs.  Per molecule (N=192, rank=4) the reference computes

    out = -V (W^T W)^{-1} W^T dDS,   W = F(V) = R V R - V

over the Gram-Schmidt basis V of the Krylov space K_4(dDS).  The
output is invariant under ANY invertible change of basis of K_4
(W is linear in V), so we use the raw power iterates S_k = R^k dDS R^k
directly:

    S_0 = D - P;  S_k = R S_{k-1} R              (8 bf16 PE products/mol)
    g[s] = <S_a, S_b>  (a+b = s, s = 0..8)       (Gram is Hankel: 9 ips)
    O[I,J] = g[I+J+2] - 2 g[I+J+1] + g[I+J],  c[J] = g[J+1] - g[J]
    y = O^{-1} c   (batched 4x4 Gauss over 32-mol blocks)
    out = -sum_I y_I S_I                         (fused scale-add chain)

Layout per pair (A, B): hi tiles [128, 384] (A rows 0:128 in cols
0:192, B in 192:384), lo tiles [128, 192] (A rows 128:192 in
partitions 0:64, B in 64:128).  Matmuls run in bf16 (PSUM fp32).
Gram inner products: DVE/Pool elementwise multiply (bf16 2x) then a
one-hot selector matmul on the PE column-sums each product into a
per-molecule row of a PSUM accumulator; a per-level tensor_reduce
lands g directly in the [32, 9] solver layout.
"""

import sys

sys.path.insert(0, "/opt/trn_rl_repo")

import numpy as np

import concourse.bass as bass
import concourse.bacc as bacc
import concourse.tile as tile
from concourse import mybir
from concourse.bass_utils import run_bass_kernel_spmd

F32 = mybir.dt.float32
BF16 = mybir.dt.bfloat16
ALU = mybir.AluOpType
ACTF = mybir.ActivationFunctionType

NMOL, N, RANK = 512, 192, 4
NCORES = 8
MPC = NMOL // NCORES      # 64 molecules per core
NPAIR = MPC // 2          # 32 pairs
BLKP = 8                  # pairs per block (16 mols -> one batched solve)
NBLK = NPAIR // BLKP
HI, LO = 128, 64
PAIR_OFF = 4 * 2 * BLKP   # ybc column offset of the packed-lo pair values

# g[s] = <S_a, S_b> with a+b = s; level k (k=1..4) computes s = 2k-1, 2k.
G_PAIRS = {0: (0, 0), 1: (0, 1), 2: (1, 1), 3: (1, 2), 4: (2, 2),
           5: (2, 3), 6: (3, 3), 7: (3, 4), 8: (4, 4)}


def build_core_kernel():
    nc = bacc.Bacc(None, target_bir_lowering=False, enable_partition_id=False)
    D = nc.dram_tensor("D", [MPC, N, N], F32, kind="ExternalInput")
    P = nc.dram_tensor("P", [MPC, N, N], F32, kind="ExternalInput")
    R = nc.dram_tensor("Rm", [MPC, N, N], F32, kind="ExternalInput")
    OUT = nc.dram_tensor("OUT", [MPC, N, N], F32, kind="ExternalOutput")
    with tile.TileContext(nc) as tc:
        _body(nc, tc, D, P, R, OUT)
    nc.finalize()
    return nc


def _consts(nc, pool):
    c = {}
    # cb1: one-hot ones-column selector bank (col 31 = all-ones, fp32);
    # window cb1[:, 31-r : 63-r] routes a partials column-sum to gather
    # row r.
    W = 2 * BLKP
    cb1 = pool.tile([HI, 2 * W - 1], F32)
    nc.vector.memset(cb1, 0.0)
    nc.vector.memset(cb1[:, W - 1:W], 1.0)
    c["cb1"] = cb1
    # cb2: col 31 = upper-half ones, col 32 = lower-half ones; window at
    # row r sends partitions 0:64 to row r and 64:128 to row r+1.
    cb2 = pool.tile([HI, 2 * W], F32)
    nc.vector.memset(cb2, 0.0)
    nc.vector.memset(cb2[0:LO, W - 1:W], 1.0)
    nc.vector.memset(cb2[LO:HI, W:W + 1], 1.0)
    c["cb2"] = cb2
    ones = pool.tile([HI, HI], F32)
    nc.vector.memset(ones, 1.0)
    c["ones"] = ones
    # selp: even partitions -> ones in cols 0:64, odd -> ones in 64:128.
    idp = pool.tile([HI, 1], mybir.dt.int32)
    nc.gpsimd.iota(idp, pattern=[[0, 1]], base=0, channel_multiplier=1)
    podd_i = pool.tile([HI, 1], mybir.dt.int32)
    nc.vector.tensor_scalar(out=podd_i, in0=idp, scalar1=1, scalar2=None,
                            op0=ALU.bitwise_and)
    podd = pool.tile([HI, 1], F32)
    nc.vector.tensor_scalar(out=podd, in0=podd_i, scalar1=1.0, scalar2=None,
                            op0=ALU.mult)
    pevn = pool.tile([HI, 1], F32)
    nc.vector.tensor_scalar(out=pevn, in0=podd, scalar1=-1.0, scalar2=1.0,
                            op0=ALU.mult, op1=ALU.add)
    selp = pool.tile([HI, HI], F32)
    nc.vector.tensor_scalar(out=selp[:, 0:LO], in0=ones[:, 0:LO],
                            scalar1=pevn, scalar2=None, op0=ALU.mult)
    nc.vector.tensor_scalar(out=selp[:, LO:HI], in0=ones[:, 0:LO],
                            scalar1=podd, scalar2=None, op0=ALU.mult)
    c["selp"] = selp
    # mask32[c, 4m+I] = (c == m); mask2[c, 4j+I] = (c in {2j, 2j+1})
    nm = 2 * BLKP
    mi = pool.tile([nm, 4 * nm], mybir.dt.int32)
    nc.gpsimd.iota(mi, pattern=[[-1, nm], [0, 4]], base=0,
                   channel_multiplier=1)
    mask32 = pool.tile([nm, 4 * nm], F32)
    nc.vector.tensor_scalar(out=mask32, in0=mi, scalar1=0, scalar2=None,
                            op0=ALU.is_equal)
    c["mask32"] = mask32
    mj = pool.tile([nm, 4 * BLKP], mybir.dt.int32)
    nc.gpsimd.iota(mj, pattern=[[-2, BLKP], [0, 4]], base=0,
                   channel_multiplier=1)
    m20 = pool.tile([nm, 4 * BLKP], F32)
    nc.vector.tensor_scalar(out=m20, in0=mj, scalar1=0, scalar2=None,
                            op0=ALU.is_equal)
    m21 = pool.tile([nm, 4 * BLKP], F32)
    nc.vector.tensor_scalar(out=m21, in0=mj, scalar1=1, scalar2=None,
                            op0=ALU.is_equal)
    mask2 = pool.tile([nm, 4 * BLKP], F32)
    nc.vector.tensor_add(mask2, m20, m21)
    c["mask2"] = mask2
    return c


def _body(nc, tc, D, P, R, OUT):
    import contextlib

    ctx = contextlib.ExitStack()
    with ctx:
        cpool = ctx.enter_context(tc.tile_pool(name="consts", bufs=1))
        sp = ctx.enter_context(tc.tile_pool(name="sp", bufs=BLKP + 2))
        tmp = ctx.enter_context(tc.tile_pool(name="tmp", bufs=3))
        tail = ctx.enter_context(tc.tile_pool(name="tail", bufs=2))
        ps_big = ctx.enter_context(tc.tile_pool(name="ps_big", bufs=3,
                                                space="PSUM"))
        ps_lo = ctx.enter_context(tc.tile_pool(name="ps_lo", bufs=3,
                                               space="PSUM"))
        ps_tl = ctx.enter_context(tc.tile_pool(name="ps_tl", bufs=1,
                                               space="PSUM"))
        C = _consts(nc, cpool)

        for b in range(NBLK):
            pairs = list(range(b * BLKP, (b + 1) * BLKP))
            st = {}
            for q in pairs:
                st[q] = _load_prep(nc, D, P, R, q, sp, tmp)
            # gram for level k-1 is emitted after level k's products so the
            # ACT squares queue behind the drains they'd otherwise delay
            for k in range(1, RANK + 1):
                _level(nc, st, pairs, k, sp, tmp, ps_big, ps_lo)
                _gram_level(nc, st, pairs,
                            [0] if k == 1 else [2 * k - 3, 2 * k - 2], tmp)
            _gram_level(nc, st, pairs, [2 * RANK - 1, 2 * RANK], tmp)
            g_sb = _gather(nc, st, pairs, C, tail, ps_tl)
            ybc = _tail(nc, pairs, C, tail, ps_tl, g_sb)
            for q in pairs:
                _combo(nc, st[q], q - b * BLKP, ybc, tmp)
            for q in pairs:
                _store(nc, OUT, q, st[q])


def _load_prep(nc, D, P, R, q, sp, tmp):
    """DMA loads, dDS = D - P (bf16), R cast for one pair."""
    mA, mB = 2 * q, 2 * q + 1
    stg = {}
    for nm, T in (("d", D), ("p", P), ("r", R)):
        sh = tmp.tile([HI, 2 * N], F32, name=f"{nm}sh", tag="stgh", bufs=10)
        sl = tmp.tile([HI, N], F32, name=f"{nm}sl", tag="stgl", bufs=10)
        nc.sync.dma_start(out=sh.rearrange("p (m c) -> p m c", m=2),
                          in_=T[mA:mA + 2, 0:HI, :].transpose([1, 0, 2]))
        nc.sync.dma_start(out=sl, in_=T[mA:mA + 2, HI:N, :])
        stg[nm] = (sh, sl)

    s0h = sp.tile([HI, 2 * N], BF16, tag="s0h")
    s0l = sp.tile([HI, N], BF16, tag="s0l")
    nc.vector.tensor_sub(s0h, stg["d"][0], stg["p"][0])
    nc.gpsimd.tensor_sub(s0l, stg["d"][1], stg["p"][1])
    rh = sp.tile([HI, 2 * N], BF16, tag="rh")
    rl = sp.tile([HI, N], BF16, tag="rl")
    nc.scalar.copy(rh, stg["r"][0])
    nc.gpsimd.tensor_copy(rl, stg["r"][1])
    rbd = sp.tile([HI, HI], BF16, tag="rbd")
    nc.gpsimd.memset(rbd, 0.0)
    nc.gpsimd.tensor_copy(rbd[0:LO, 0:LO], rl[0:LO, HI:N])
    nc.gpsimd.tensor_copy(rbd[LO:HI, LO:HI], rl[LO:HI, HI:N])
    partials = sp.tile([HI, 27], F32, tag="part")
    return {"sh": [s0h], "sl": [s0l], "rh": rh, "rl": rl, "rbd": rbd,
            "part": partials}


def _mm_pair(nc, ps_big, ps_lo, lhs_hi, lhs_lo, bd, rhs_hi, rhs_lo, tagp):
    """One 192x192 @ 192x192 product for both pair mols -> PSUM pair tiles.

    out[p,f] = sum_c lhs[c,p] rhs[c,f] per molecule; lhs must be symmetric
    (we pass S or R directly as lhsT).  bd is the block-diagonal packing of
    the two mols' (c-lo, p-lo) corner chunks so that corner runs as ONE
    matmul over the packed-lo partitions.
    """
    ph = ps_big.tile([HI, 2 * N], F32, name=f"ph_{tagp}", tag="pbig")
    pl = ps_lo.tile([HI, N], F32, name=f"pl_{tagp}", tag="plo")
    for m, c0, p0 in ((0, 0, 0), (1, N, LO)):  # mol A, mol B
        hi_c = lhs_hi[:, c0:c0 + HI]          # lhs cols 0:128 (out rows hi)
        hi_cl = lhs_hi[:, c0 + HI:c0 + N]     # lhs cols 128:192 (out rows lo)
        lo_c = lhs_lo[p0:p0 + LO, 0:HI]
        rhi = rhs_hi[:, c0:c0 + N]
        rlo = rhs_lo[p0:p0 + LO, :]
        nc.tensor.matmul(ph[:, c0:c0 + N], lhsT=hi_c, rhs=rhi,
                         start=True, stop=False)
        nc.tensor.matmul(ph[:, c0:c0 + N], lhsT=lo_c, rhs=rlo,
                         start=False, stop=True)
        nc.tensor.matmul(pl[p0:p0 + LO, :], lhsT=hi_cl, rhs=rhi,
                         start=True, stop=False)
    nc.tensor.matmul(pl[:, :], lhsT=bd, rhs=rhs_lo,
                     start=False, stop=True, skip_group_check=True)
    return ph, pl


def _drain(nc, eng, out, in_):
    if eng == "dve":
        nc.vector.tensor_copy(out, in_)
    else:
        nc.scalar.copy(out, in_)


# engine schedule for the per-level PSUM drains (dve/act only: gpsimd
# cannot touch PSUM)
T_BIG_ENG = ["act", "act", "act", "act"]
S_BIG_ENG = ["act", "act", "act", "act"]
T_SM_ENG = ["dve", "dve", "dve", "dve"]
S_SM_ENG = ["dve", "dve", "dve", "dve"]


def _level(nc, st, pairs, k, sp, tmp, ps_big, ps_lo):
    """Level k: T = S_{k-1} R then S_k = R T for every pair."""
    bds = {}
    for q in pairs:
        s = st[q]
        bd = tmp.tile([HI, HI], BF16, name=f"bdt{k}_{q}", tag="bdt",
                      bufs=BLKP + 2)
        sl = s["sl"][k - 1]
        nc.gpsimd.memset(bd, 0.0)
        nc.gpsimd.tensor_copy(bd[0:LO, 0:LO], sl[0:LO, HI:N])
        nc.gpsimd.tensor_copy(bd[LO:HI, LO:HI], sl[LO:HI, HI:N])
        bds[q] = bd
    tps = {}
    for q in pairs:
        s = st[q]
        tps[q] = _mm_pair(nc, ps_big, ps_lo, s["sh"][k - 1], s["sl"][k - 1],
                          bds[q], s["rh"], s["rl"], f"t{k}_{q}")
    tts = {}
    for q in pairs:
        th = tmp.tile([HI, 2 * N], BF16, name=f"th{k}_{q}", tag="th",
                      bufs=BLKP + 2)
        tl = tmp.tile([HI, N], BF16, name=f"tl{k}_{q}", tag="tl",
                      bufs=BLKP + 2)
        _drain(nc, T_BIG_ENG[k - 1], th, tps[q][0])
        _drain(nc, T_SM_ENG[k - 1], tl, tps[q][1])
        tts[q] = (th, tl)
    sps = {}
    for q in pairs:
        s = st[q]
        sps[q] = _mm_pair(nc, ps_big, ps_lo, s["rh"], s["rl"], s["rbd"],
                          tts[q][0], tts[q][1], f"s{k}_{q}")
    for q in pairs:
        s = st[q]
        skh = sp.tile([HI, 2 * N], BF16, name=f"s{k}h", tag=f"s{k}h",
                      bufs=BLKP + 2)
        skl = sp.tile([HI, N], BF16, name=f"s{k}l", tag=f"s{k}l",
                      bufs=BLKP + 2)
        _drain(nc, S_BIG_ENG[k - 1], skh, sps[q][0])
        _drain(nc, S_SM_ENG[k - 1], skl, sps[q][1])
        s["sh"].append(skh)
        s["sl"].append(skl)


# evens (squares) hi tiles go to ACT; everything else is DVE TTR
def _gram_level(nc, st, pairs, svals, tmp):
    """g[s] = <S_a, S_b> partial sums into partials cols (hi-A: s,
    hi-B: 9+s, lo-pair: 18+s) via fused TTR on DVE / Square-accum on ACT."""
    for s in svals:
        a, bb = G_PAIRS[s]
        for q in pairs:
            stq = st[q]
            part = stq["part"]
            ah, bh = stq["sh"][a], stq["sh"][bb]
            al, bl = stq["sl"][a], stq["sl"][bb]
            if a == bb:
                for m, col in ((0, s), (1, 9 + s)):
                    junk = tmp.tile([HI, N], BF16, name="ja", tag="ja",
                                    bufs=3)
                    nc.scalar.activation(out=junk,
                                         in_=ah[:, m * N:(m + 1) * N],
                                         func=ACTF.Square,
                                         accum_out=part[:, col:col + 1])
            else:
                for m, col in ((0, s), (1, 9 + s)):
                    junk = tmp.tile([HI, N], BF16, name="jd", tag="jd",
                                    bufs=3)
                    nc.vector.scalar_tensor_tensor(
                        out=junk, in0=ah[:, m * N:(m + 1) * N], scalar=1.0,
                        in1=bh[:, m * N:(m + 1) * N],
                        op0=ALU.mult, op1=ALU.mult,
                        accum_out=part[:, col:col + 1])
            junk = tmp.tile([HI, N], BF16, name="jl", tag="jd", bufs=3)
            nc.vector.scalar_tensor_tensor(
                out=junk, in0=al, scalar=1.0, in1=bl,
                op0=ALU.mult, op1=ALU.mult, accum_out=part[:, 18 + s:19 + s])


def _gather(nc, st, pairs, C, tail, ps_tl):
    """Cross-partition reduce all pairs' partials into g_sb [32, 9]."""
    cb1, cb2 = C["cb1"], C["cb2"]
    gath = ps_tl.tile([2 * BLKP, 12], F32, tag="gath")
    nmm = 3 * len(pairs)
    i = 0
    for j, q in enumerate(pairs):
        part = st[q]["part"]
        rA = 2 * j
        W = 2 * BLKP
        for lhsT, rhs in (
            (cb1[:, W - 1 - rA:2 * W - 1 - rA], part[:, 0:9]),
            (cb1[:, W - 2 - rA:2 * W - 2 - rA], part[:, 9:18]),
            (cb2[:, W - 1 - rA:2 * W - 1 - rA], part[:, 18:27]),
        ):
            nc.tensor.matmul(gath[:, 0:9], lhsT=lhsT, rhs=rhs,
                             start=(i == 0), stop=(i == nmm - 1))
            i += 1
    g_sb = tail.tile([2 * BLKP, 9], F32, tag="g_sb")
    nc.vector.tensor_copy(g_sb, gath[:, 0:9])
    return g_sb


def _tail(nc, pairs, C, tail, ps_tl, g_sb):
    """Batched 4x4 solve from g, then broadcast -y to [128, *] columns."""
    nm = 2 * BLKP  # 32 molecules
    g = g_sb
    # Hankel assembly: h[s] = g[s] - 2 g[s+1] + g[s+2]; rhs c = diff(g)
    hs = tail.tile([nm, 7], F32, tag="hs")
    hm = tail.tile([nm, 7], F32, tag="hm")
    h = tail.tile([nm, 7], F32, tag="h")
    nc.vector.tensor_add(hs, g[:, 0:7], g[:, 2:9])
    nc.vector.tensor_scalar(out=hm, in0=g[:, 1:8], scalar1=-2.0, scalar2=None,
                            op0=ALU.mult)
    nc.vector.tensor_add(h, hs, hm)
    sv = tail.tile([nm, 14], F32, tag="sv")
    nc.vector.tensor_copy(sv[:, 0:4], h[:, 0:4])
    nc.vector.tensor_copy(sv[:, 4:7], h[:, 2:5])
    nc.vector.tensor_copy(sv[:, 7:9], h[:, 4:6])
    nc.vector.tensor_copy(sv[:, 9:10], h[:, 6:7])
    nc.vector.tensor_sub(sv[:, 10:14], g[:, 1:5], g[:, 0:4])

    ysb = _solve(nc, sv, tail, nm)
    ysn = tail.tile([nm, 4], F32, tag="ysn")
    nc.vector.tensor_scalar(out=ysn, in0=ysb, scalar1=-1.0, scalar2=None,
                            op0=ALU.mult)

    # broadcast -y to all partitions: cols 4m:(4m+4) per mol; cols
    # 128+4q:(128+4q+4) carry the packed-lo per-partition-half values.
    ones, selp = C["ones"], C["selp"]
    ysn_b = ysn.unsqueeze(1)
    yp = tail.tile([nm, 4 * nm], F32, tag="yp")
    nc.vector.tensor_mul(
        yp.rearrange("p (m i) -> p m i", i=4),
        C["mask32"].rearrange("p (m i) -> p m i", i=4),
        ysn_b.broadcast_to([nm, nm, 4]))
    yq = tail.tile([nm, 4 * BLKP], F32, tag="yq")
    nc.vector.tensor_mul(
        yq.rearrange("p (m i) -> p m i", i=4),
        C["mask2"].rearrange("p (m i) -> p m i", i=4),
        ysn_b.broadcast_to([nm, BLKP, 4]))
    ybp = ps_tl.tile([HI, N], F32, tag="ybp")
    nc.tensor.matmul(ybp[:, 0:4 * nm], lhsT=ones[0:nm, 0:HI], rhs=yp,
                     start=True, stop=True)
    nc.tensor.matmul(ybp[:, 4 * nm:4 * nm + 4 * BLKP],
                     lhsT=selp[0:nm, 0:HI],
                     rhs=yq, start=True, stop=True)
    ybc = tail.tile([HI, N], F32, tag="ybc")
    nc.scalar.copy(ybc, ybp)
    return ybc


def _solve(nc, sv, tail, nm):
    """Batched symmetric 4x4 Gauss elimination on [nm,1] column APs.

    sv cols: 0:a 1:b 2:c 3:d | 4:e 5:f 6:g | 7:h 8:i | 9:j | 10..13 r0..r3.
    Mirrors solve_batched_np (validated offline).
    """
    pp = tail.tile([nm, 4], F32, tag="pp")
    l3 = tail.tile([nm, 3], F32, tag="l3")
    tt = tail.tile([nm, 3], F32, tag="tt")
    ysb = tail.tile([nm, 4], F32, tag="ysb")

    ts = nc.vector.tensor_scalar
    sub = nc.vector.tensor_sub
    rec = nc.vector.reciprocal

    def upd(dst, src, scal, w=1):
        ts(out=tt[:, 0:w], in0=src, scalar1=scal, scalar2=None, op0=ALU.mult)
        sub(dst, dst, tt[:, 0:w])

    rec(pp[:, 0:1], sv[:, 0:1])
    ts(out=l3, in0=sv[:, 1:4], scalar1=pp[:, 0:1], scalar2=None, op0=ALU.mult)
    upd(sv[:, 4:7], l3, sv[:, 1:2], 3)          # (e,f,g) -= l*b
    upd(sv[:, 7:9], l3[:, 1:3], sv[:, 2:3], 2)  # (h,i) -= (l2,l3)*c
    upd(sv[:, 9:10], l3[:, 2:3], sv[:, 3:4])    # j -= l3*d
    upd(sv[:, 11:14], l3, sv[:, 10:11], 3)      # (r1,r2,r3) -= l*r0
    rec(pp[:, 1:2], sv[:, 4:5])
    ts(out=l3[:, 1:3], in0=sv[:, 5:7], scalar1=pp[:, 1:2], scalar2=None,
       op0=ALU.mult)
    upd(sv[:, 7:9], l3[:, 1:3], sv[:, 5:6], 2)
    upd(sv[:, 9:10], l3[:, 2:3], sv[:, 6:7])
    upd(sv[:, 12:14], l3[:, 1:3], sv[:, 11:12], 2)
    rec(pp[:, 2:3], sv[:, 7:8])
    ts(out=l3[:, 2:3], in0=sv[:, 8:9], scalar1=pp[:, 2:3], scalar2=None,
       op0=ALU.mult)
    upd(sv[:, 9:10], l3[:, 2:3], sv[:, 8:9])
    upd(sv[:, 13:14], l3[:, 2:3], sv[:, 12:13])
    rec(pp[:, 3:4], sv[:, 9:10])
    ts(out=ysb[:, 3:4], in0=sv[:, 13:14], scalar1=pp[:, 3:4], scalar2=None,
       op0=ALU.mult)
    upd(sv[:, 12:13], sv[:, 8:9], ysb[:, 3:4])
    ts(out=ysb[:, 2:3], in0=sv[:, 12:13], scalar1=pp[:, 2:3], scalar2=None,
       op0=ALU.mult)
    upd(sv[:, 11:12], sv[:, 5:6], ysb[:, 2:3])
    upd(sv[:, 11:12], sv[:, 6:7], ysb[:, 3:4])
    ts(out=ysb[:, 1:2], in0=sv[:, 11:12], scalar1=pp[:, 1:2], scalar2=None,
       op0=ALU.mult)
    upd(sv[:, 10:11], sv[:, 1:2], ysb[:, 1:2])
    upd(sv[:, 10:11], sv[:, 2:3], ysb[:, 2:3])
    upd(sv[:, 10:11], sv[:, 3:4], ysb[:, 3:4])
    ts(out=ysb[:, 0:1], in0=sv[:, 10:11], scalar1=pp[:, 0:1], scalar2=None,
       op0=ALU.mult)
    return ysb


def _combo(nc, stq, j, ybc, tmp):
    """acc = sum_I (-y_I) S_I via a fused scale-accumulate (STT) chain."""
    mA, mB = 2 * j, 2 * j + 1
    ah = tmp.tile([HI, 2 * N], F32, name="acch", tag="acch", bufs=4)
    al = tmp.tile([HI, N], F32, name="accl", tag="accl", bufs=4)
    for m, c0 in ((mA, 0), (mB, N)):
        u0 = tmp.tile([HI, N], BF16, name="cu0", tag="cu", bufs=6)
        nc.vector.tensor_scalar(out=u0, in0=stq["sh"][0][:, c0:c0 + N],
                                scalar1=ybc[:, 4 * m:4 * m + 1],
                                scalar2=None, op0=ALU.mult)
        for I in (1, 2):
            u1 = tmp.tile([HI, N], BF16, name="cu1", tag="cu", bufs=6)
            nc.vector.scalar_tensor_tensor(
                out=u1, in0=stq["sh"][I][:, c0:c0 + N],
                scalar=ybc[:, 4 * m + I:4 * m + I + 1], in1=u0,
                op0=ALU.mult, op1=ALU.add)
            u0 = u1
        nc.vector.scalar_tensor_tensor(
            out=ah[:, c0:c0 + N], in0=stq["sh"][3][:, c0:c0 + N],
            scalar=ybc[:, 4 * m + 3:4 * m + 4], in1=u0,
            op0=ALU.mult, op1=ALU.add)
    u0 = tmp.tile([HI, N], BF16, name="cul0", tag="cu", bufs=6)
    nc.vector.tensor_scalar(out=u0, in0=stq["sl"][0],
                            scalar1=ybc[:, PAIR_OFF + 4 * j:PAIR_OFF + 4 * j + 1],
                            scalar2=None, op0=ALU.mult)
    for I in (1, 2):
        u1 = tmp.tile([HI, N], BF16, name="cul1", tag="cu", bufs=6)
        nc.vector.scalar_tensor_tensor(
            out=u1, in0=stq["sl"][I],
            scalar=ybc[:, PAIR_OFF + 4 * j + I:PAIR_OFF + 4 * j + I + 1], in1=u0,
            op0=ALU.mult, op1=ALU.add)
        u0 = u1
    nc.vector.scalar_tensor_tensor(
        out=al, in0=stq["sl"][3],
        scalar=ybc[:, PAIR_OFF + 4 * j + 3:PAIR_OFF + 4 * j + 4], in1=u0,
        op0=ALU.mult, op1=ALU.add)
    stq["acc"] = (ah, al)


def _store(nc, OUT, q, stq):
    mA = 2 * q
    ah, al = stq["acc"]
    nc.sync.dma_start(out=OUT[mA:mA + 2, 0:HI, :].transpose([1, 0, 2]),
                      in_=ah.rearrange("p (m c) -> p m c", m=2))
    nc.sync.dma_start(out=OUT[mA:mA + 2, HI:N, :], in_=al)


_NC_CACHE = None


def _get_nc():
    global _NC_CACHE
    if _NC_CACHE is None:
        _NC_CACHE = build_core_kernel()
    return _NC_CACHE


def kernel(D, P, R, max_rank=4, _trace=False):
    D = np.ascontiguousarray(D, dtype=np.float32)
    P = np.ascontiguousarray(P, dtype=np.float32)
    R = np.ascontiguousarray(R, dtype=np.float32)
    nc = _get_nc()
    in_maps = []
    for i in range(NCORES):
        sl = slice(i * MPC, (i + 1) * MPC)
        in_maps.append({"D": D[sl], "P": P[sl], "Rm": R[sl]})
    res = run_bass_kernel_spmd(nc, in_maps, core_ids=list(range(NCORES)),
                               trace=_trace)
    out = np.concatenate([r["OUT"] for r in res.results], axis=0)
    if _trace:
        kernel.last_exec_time_ns = res.exec_time_ns
        kernel.last_trace = res.instructions_and_trace
    return out


if __name__ == "__main__":
    import tempfile
    from concourse.bass_utils import compile_bass_kernel
    nc = build_core_kernel()
    print("build OK")
    if "--compile" in sys.argv:
        td = tempfile.mkdtemp()
        print("NEFF:", compile_bass_kernel(nc, td))
